# revision 1
# baseline (speedup 1.0000x reference)
"""CFD-GCN Trainium2 kernel: 6-layer GCN + KNN-interpolate on 8 NeuronCores.

Strategy (node sharding, feature-major residency):
  - Fine nodes are sharded 6250/core (padded to 6272 = 49*128).
  - Each GCN layer l: z = h @ W_l (dense, per-core shard, PE),
    AllGather(z) -> z_full, per-edge gather of source rows (indirect DMA),
    scatter-add via one-hot matmuls into PSUM per 128-dest tile,
    bias + (relu) on ScalarE -> next h (kept feature-major in SBUF,
    which makes the next dense matmul transpose-free).
  - First layer runs sparse-first on the 6-wide input ((A h0) W0), last
    layer dense-first on the 3-wide output (A (h W5)), so only 4 of the 6
    layers move 512-wide data through the AllGather+gather path.
  - Edges are sorted by destination on the host; each 128-dest tile gets
    a fixed number KB of 128-edge blocks (padded with zero-norm slots).
    The one-hot S[e, d] = norm_e * (col_e == d) is built on DVE from a
    static iota matrix, so scatter-add = sum_j one-hot matmuls
    accumulated in PSUM.
  - KNN-interpolate: -d2 = 2 f.c - |c|^2 - |f|^2 via a K=4 matmul against
    all 2000 coarse nodes, DVE max8/max_index for the top-3, inverse-d2
    weights, tiny indirect gather of coarse_y rows.
"""

import math
import numpy as np

# ---------------------------------------------------------------- constants
N_FINE = 50000
N_COARSE = 2000
HID = 512
OUT = 3
NCORES = 8
P = 128

_PROGRAM_CACHE = {}


# ---------------------------------------------------------------- host side
def _wrap16(flat, P=128):
    """int16 index list -> dma_gather layout [P, len/16] (wrapped in 16
    partitions, replicated across the 8 Q7 cores)."""
    L = len(flat) // 16
    w = np.asarray(flat, np.int16).reshape(L, 16).T  # [16, L]
    return np.tile(w, (P // 16, 1))


def _preprocess_edges(edge_index, n_fine, ncores):
    """Sort edges by destination, shard by dest core, tile dests by 128.

    Edges of each (core, dest-tile) are split by source half (table rows
    0..HALF-1 vs HALF..), each half padded to KBA/KBB 128-edge blocks.

    Returns (KBA, KBB, nt, padsh, per-core dict arrays).
    """
    nsh = n_fine // ncores
    nt = math.ceil(nsh / P)
    padsh = nt * P
    half = (ncores // 2) * padsh

    row = np.asarray(edge_index[0]).astype(np.int64)
    col = np.asarray(edge_index[1]).astype(np.int64)
    loop = np.arange(n_fine, dtype=np.int64)
    row = np.concatenate([row, loop])
    col = np.concatenate([col, loop])

    deg = np.bincount(col, minlength=n_fine).astype(np.float32)
    dis = 1.0 / np.sqrt(deg)
    normv = (dis[row] * dis[col]).astype(np.float32)

    srcpad = ((row // nsh) * padsh + (row % nsh)).astype(np.int64)

    order = np.argsort(col, kind="stable")
    col_s, norm_s, srcpad_s = col[order], normv[order], srcpad[order]

    tiles = []
    KBA = KBB = 1
    for c in range(ncores):
        base = c * nsh
        for t in range(nt):
            lo, hi = base + t * P, min(base + (t + 1) * P, base + nsh)
            a = np.searchsorted(col_s, lo, "left")
            b = np.searchsorted(col_s, hi, "left")
            isa = srcpad_s[a:b] < half
            na, nb = int(isa.sum()), int((~isa).sum())
            tiles.append((c, t, a, b, isa, na, nb))
            KBA = max(KBA, math.ceil(max(na, 1) / P))
            KBB = max(KBB, math.ceil(max(nb, 1) / P))

    KBT = KBA + KBB
    out = []
    for c in range(ncores):
        out.append({
            "idxA": np.full((P, nt * KBA * 8), -1, np.int16),
            "idxB": np.full((P, nt * KBB * 8), -1, np.int16),
            "cntAB": np.ones((2, nt), np.int32),
            "ecol": np.zeros((P, nt * KBT), np.float32),
            "enorm": np.zeros((P, nt * KBT), np.float32),
        })
    for c, t, a, b, isa, na, nb in tiles:
        oc = out[c]
        colrel = (col_s[a:b] - (c * nsh + t * P)).astype(np.float32)
        nrm = norm_s[a:b]
        sp = srcpad_s[a:b]
        for half_i, mask, KBh, key, boff in (
                (0, isa, KBA, "idxA", 0), (1, ~isa, KBB, "idxB", KBA)):
            nh = int(mask.sum())
            flat = np.full(KBh * P, -1, np.int64)
            flat[:nh] = sp[mask] - (half if half_i else 0)
            if nh == 0:
                flat[0] = 0  # dummy valid row, zero norm
            oc[key][:, t * KBh * 8:(t + 1) * KBh * 8] = _wrap16(flat)
            oc["cntAB"][half_i, t] = max(nh, 1)
            s = np.arange(nh)
            oc["ecol"][s % P, t * KBT + boff + s // P] = colrel[mask]
            oc["enorm"][s % P, t * KBT + boff + s // P] = nrm[mask]
    return KBA, KBB, nt, padsh, out


def _pad_shard(x, nsh, padsh, ncores):
    d = x.shape[1]
    out = np.zeros((ncores * padsh, d), x.dtype)
    for c in range(ncores):
        out[c * padsh : c * padsh + nsh] = x[c * nsh : (c + 1) * nsh]
    return out


# ---------------------------------------------------------------- device side
def build_program(n_fine, n_coarse, hid, out_dim, ncores, KBA, KBB, nt):
    import concourse.bass as bass
    import concourse.mybir as mybir
    from concourse.bacc import Bacc
    from concourse.tile import TileContext
    from concourse.masks import make_identity
    from contextlib import ExitStack

    F32 = mybir.dt.float32
    BF16 = mybir.dt.bfloat16
    I32 = mybir.dt.int32
    padsh = nt * P
    npad = ncores * padsh
    half = (ncores // 2) * padsh
    kc = hid // P
    KBT = KBA + KBB
    nblk = nt * KBT
    rg = [list(range(ncores))]
    AF = mybir.ActivationFunctionType
    ALU = mybir.AluOpType
    IOO = bass.IndirectOffsetOnAxis
    ncpad = math.ceil(n_coarse / 512) * 512
    ncc = math.ceil(n_coarse / 512)

    nc = Bacc(num_devices=ncores)

    # ---- kernel I/O (per core) ----
    I16 = mybir.dt.int16
    h0 = nc.declare_dram_parameter("h0", [npad, 64], F32, isOutput=False)
    idxA = nc.declare_dram_parameter("idxA", [P, nt * KBA * 8], I16, isOutput=False)
    idxB = nc.declare_dram_parameter("idxB", [P, nt * KBB * 8], I16, isOutput=False)
    cntAB = nc.declare_dram_parameter("cntAB", [2, nt], I32, isOutput=False)
    ecol = nc.declare_dram_parameter("ecol", [P, nblk], F32, isOutput=False)
    enorm = nc.declare_dram_parameter("enorm", [P, nblk], F32, isOutput=False)
    xposT = nc.declare_dram_parameter("xposT", [2, padsh], F32, isOutput=False)
    xpos_nm = nc.declare_dram_parameter("xpos_nm", [padsh, 2], F32, isOutput=False)
    coarseT = nc.declare_dram_parameter("coarseT", [2, n_coarse], F32, isOutput=False)
    ycoarse = nc.declare_dram_parameter("ycoarse", [n_coarse, out_dim], F32, isOutput=False)
    w_mid = [nc.declare_dram_parameter(n, [hid, hid], F32, isOutput=False)
             for n in ("w1", "w2", "we0", "we1")]
    b_mid = [nc.declare_dram_parameter(n, [hid], F32, isOutput=False)
             for n in ("b1", "b2", "be0", "be1")]
    w0 = nc.declare_dram_parameter("w0", [6, hid], F32, isOutput=False)
    b0 = nc.declare_dram_parameter("b0", [hid], F32, isOutput=False)
    wtop = nc.declare_dram_parameter("wtop", [out_dim, hid], F32, isOutput=False)
    w5 = nc.declare_dram_parameter("w5", [hid, out_dim], F32, isOutput=False)
    b5 = nc.declare_dram_parameter("b5", [out_dim], F32, isOutput=False)
    y_out = nc.declare_dram_parameter("out", [padsh, out_dim], F32, isOutput=True)

    # ---- internal DRAM ----
    zsh = [nc.dram_tensor(f"zsh{i}", [padsh, hid], F32) for i in range(4)]
    zfull = [nc.dram_tensor(f"zfull{i}", [npad, hid], F32, addr_space="Shared")
             for i in range(4)]
    z5sh = nc.dram_tensor("z5sh", [padsh, 64], F32)
    z5full = nc.dram_tensor("z5full", [npad, 64], F32, addr_space="Shared")

    with TileContext(nc) as tc:
        with ExitStack() as ctx:
            main = ctx.enter_context(tc.tile_pool(name="main", bufs=1))
            wpool = ctx.enter_context(tc.tile_pool(name="wpool", bufs=2))
            sp = ctx.enter_context(tc.tile_pool(name="sp", bufs=KBT + 2))
            zp = ctx.enter_context(tc.tile_pool(name="zp", bufs=2))
            smallp = ctx.enter_context(tc.tile_pool(name="smallp", bufs=2))
            # PSUM: three pools, one shared tag each -> 6 banks max
            ppA = ctx.enter_context(tc.tile_pool(name="ppA", bufs=2, space="PSUM"))
            ppB = ctx.enter_context(tc.tile_pool(name="ppB", bufs=2, space="PSUM"))
            ppC = ctx.enter_context(tc.tile_pool(name="ppC", bufs=2, space="PSUM"))

            def accps(shape):
                return ppA.tile(shape, F32, tag="acc", name="acc")

            def densps(shape):
                return ppB.tile(shape, F32, tag="dacc", name="dacc")

            def tps(shape):
                return ppC.tile(shape, F32, tag="tp", name="tp")

            # ---------- persistent tiles ----------
            hT = main.tile([P, kc, padsh], BF16, tag="hT")
            y3n = main.tile([P, nt, out_dim], F32, tag="y3n")  # node-major top3 result
            iota_f = main.tile([P, P], F32, tag="iota_f")
            iden = main.tile([P, P], F32, tag="iden")
            idxA_sb = main.tile([P, nt * KBA * 8], I16, tag="idxA_sb")
            idxB_sb = main.tile([P, nt * KBB * 8], I16, tag="idxB_sb")
            cnt_sb = main.tile([2, nt], I32, tag="cnt_sb")
            ecol_sb = main.tile([P, nblk], F32, tag="ecol_sb")
            enorm_sb = main.tile([P, nblk], F32, tag="enorm_sb")
            wtop_sb = main.tile([out_dim, hid], F32, tag="wtop_sb")

            nc.sync.dma_start(out=idxA_sb[:], in_=idxA[:, :])
            nc.sync.dma_start(out=idxB_sb[:], in_=idxB[:, :])
            nc.sync.dma_start(out=cnt_sb[:], in_=cntAB[:, :])
            nc.sync.dma_start(out=ecol_sb[:], in_=ecol[:, :])
            nc.sync.dma_start(out=enorm_sb[:], in_=enorm[:, :])
            nc.sync.dma_start(out=wtop_sb[:], in_=wtop[:, :])

            iota_i = smallp.tile([P, P], I32, tag="iota_i")
            nc.gpsimd.iota(out=iota_i[:], pattern=[[1, P]], base=0, channel_multiplier=0)
            nc.vector.tensor_copy(out=iota_f[:], in_=iota_i[:])
            make_identity(nc, iden[:])

            # ---------- helpers ----------
            def load_w_mid(wd):
                # SWDGE casts f32 -> bf16 during the DMA
                w_sb = wpool.tile([P, kc, hid], BF16, tag="w_sb")
                nc.gpsimd.dma_start(
                    out=w_sb[:], in_=wd[:, :].rearrange("(k p) h -> p k h", p=P))
                return w_sb

            def load_b_mid(bd):
                b_sb = wpool.tile([P, kc], F32, tag="b_sb")
                nc.sync.dma_start(out=b_sb[:], in_=bd[:].rearrange("(k p) -> p k", p=P))
                return b_sb

            _regctr = [0]

            def edge_gather(msg, t, tableA, tableB, elem):
                # half A -> blocks [0, KBA), half B -> blocks [KBA, KBT)
                for (tab, idx_sb, KBh, boff, hrow) in (
                        (tableA, idxA_sb, KBA, 0, 0),
                        (tableB, idxB_sb, KBB, KBA, 1)):
                    _regctr[0] += 1
                    r = nc.gpsimd.alloc_register(f"gcnt{_regctr[0]}")
                    nc.gpsimd.reg_load(r, cnt_sb[hrow:hrow + 1, t:t + 1])
                    nc.gpsimd.dma_gather(
                        msg[:, boff:boff + KBh, :], tab,
                        idx_sb[:, t * KBh * 8:(t + 1) * KBh * 8],
                        KBh * P, r, elem)

            # persistent ping-pong gather buffers (fixed addresses, zeroed
            # once: -1-skipped slots must read as finite; their norms are 0)
            msg_pp = [main.tile([P, KBT, hid], F32, tag=f"msgpp{i}", name="msgpp")
                      for i in range(2)]
            msg6_pp = [main.tile([P, KBT, 64], F32, tag=f"msg6pp{i}", name="msg6pp")
                       for i in range(2)]
            z5w_pp = [main.tile([P, 64], F32, tag=f"z5wpp{i}", name="z5wpp")
                      for i in range(2)]
            for m in (*msg_pp, *msg6_pp, *z5w_pp):
                nc.gpsimd.memset(m[:], 0.0)

            def make_S(g):
                S = sp.tile([P, P], F32, tag="S")
                nc.vector.tensor_scalar(
                    out=S[:], in0=iota_f[:],
                    scalar1=ecol_sb[:, g:g + 1], scalar2=enorm_sb[:, g:g + 1],
                    op0=ALU.is_equal, op1=ALU.mult)
                return S

            # ---------- KNN (independent; writes y3n) ----------
            # -d2[m, n] = 2 f_m . c_n - |c_n|^2 - |f_m|^2 :
            #   matmul K=3 with lhsT rows [2fx, 2fy, -1], rhs rows
            #   [cx, cy, |c|^2], then a per-partition add of -|f_m|^2.
            with tc.tile_pool(name="knn", bufs=2) as kp:
                mones_sb = kp.tile([1, P], F32, tag="mones_sb", bufs=1)
                nc.vector.memset(mones_sb[:], -1.0)
                coarse3 = kp.tile([3, n_coarse], F32, tag="coarse3", bufs=1)
                with tc.tile_pool(name="knnprep", bufs=1) as kprep:
                    nc.sync.dma_start(out=coarse3[0:2, :], in_=coarseT[:, :])
                    pones = kprep.tile([2, 1], F32, tag="pones")
                    nc.vector.memset(pones[:], 1.0)
                    csq = kprep.tile([1, n_coarse], F32, tag="csq")
                    for i in range(ncc):
                        a, b = i * 512, min((i + 1) * 512, n_coarse)
                        sqc = kprep.tile([2, 512], F32, tag="sqc")
                        nc.vector.tensor_tensor(out=sqc[:, : b - a],
                                                in0=coarse3[0:2, a:b],
                                                in1=coarse3[0:2, a:b], op=ALU.mult)
                        ps = tps([P, 512])
                        nc.tensor.matmul(out=ps[0:1, : b - a], lhsT=pones[:],
                                         rhs=sqc[:, : b - a], start=True, stop=True)
                        nc.vector.tensor_copy(out=csq[:, a:b], in_=ps[0:1, : b - a])
                    # row 2 (|c|^2) via DMA (compute engines can't start at
                    # partition 2)
                    nc.sync.dma_start(out=coarse3[2:3, :], in_=csq[:])

                    # -|f|^2 per node, node-major: [P, nt]
                    xnm = kprep.tile([P, nt, 2], F32, tag="xnm")
                    nc.sync.dma_start(
                        out=xnm[:], in_=xpos_nm[:, :].rearrange("(t p) d -> p t d", p=P))
                    sqn = kprep.tile([P, nt, 2], F32, tag="sqn")
                    nc.vector.tensor_tensor(out=sqn[:], in0=xnm[:], in1=xnm[:],
                                            op=ALU.mult)
                    fsqneg = kp.tile([P, nt], F32, tag="fsqneg", bufs=1)
                    nc.vector.tensor_reduce(out=fsqneg[:], in_=sqn[:],
                                            axis=mybir.AxisListType.X, op=ALU.add,
                                            negate=True)

                for t in range(nt):
                    tp = t * P
                    xp_t = kp.tile([2, P], F32, tag="xp_t")
                    nc.sync.dma_start(out=xp_t[:], in_=xposT[:, tp:tp + P])
                    lhsT3 = kp.tile([3, P], F32, tag="lhsT3")
                    nc.vector.tensor_scalar_mul(lhsT3[0:2, :], xp_t[:], 2.0)
                    nc.sync.dma_start(out=lhsT3[2:3, :], in_=mones_sb[:])

                    d2 = kp.tile([P, ncpad], F32, tag="d2", bufs=1)
                    for i in range(ncc):
                        a, b = i * 512, min((i + 1) * 512, n_coarse)
                        dps = densps([P, 512])
                        nc.tensor.matmul(out=dps[:, : b - a], lhsT=lhsT3[:],
                                         rhs=coarse3[:, a:b], start=True, stop=True)
                        nc.vector.tensor_scalar(out=d2[:, a:b], in0=dps[:, : b - a],
                                                scalar1=fsqneg[:, t:t + 1],
                                                scalar2=None, op0=ALU.add)
                    vals = kp.tile([P, 8], F32, tag="vals")
                    nc.vector.max(out=vals[:], in_=d2[:, 0:n_coarse])
                    idxs = kp.tile([P, 8], mybir.dt.uint32, tag="idxs")
                    nc.vector.max_index(out=idxs[:], in_max=vals[:],
                                        in_values=d2[:, 0:n_coarse])
                    wv = kp.tile([P, 3], F32, tag="wv")
                    nc.vector.tensor_scalar(out=wv[:], in0=vals[:, 0:3],
                                            scalar1=-1.0, scalar2=1e-16,
                                            op0=ALU.mult, op1=ALU.max)
                    nc.vector.reciprocal(out=wv[:], in_=wv[:])
                    wsum = kp.tile([P, 1], F32, tag="wsum")
                    nc.vector.tensor_reduce(out=wsum[:], in_=wv[:],
                                            axis=mybir.AxisListType.X, op=ALU.add)
                    nc.vector.reciprocal(out=wsum[:], in_=wsum[:])
                    nc.vector.tensor_scalar(out=wv[:], in0=wv[:],
                                            scalar1=wsum[:, 0:1], scalar2=None,
                                            op0=ALU.mult)
                    yg = kp.tile([P, 3, out_dim], F32, tag="yg")
                    for k3 in range(3):
                        nc.gpsimd.indirect_dma_start(
                            out=yg[:, k3, :], out_offset=None, in_=ycoarse[:, :],
                            in_offset=IOO(ap=idxs[:, k3:k3 + 1], axis=0))
                    tmp = kp.tile([P, out_dim], F32, tag="tmp")
                    nc.vector.tensor_scalar(out=y3n[:, t, :], in0=yg[:, 0, :],
                                            scalar1=wv[:, 0:1], scalar2=None,
                                            op0=ALU.mult)
                    for k in (1, 2):
                        nc.vector.tensor_scalar(out=tmp[:], in0=yg[:, k, :],
                                                scalar1=wv[:, k:k + 1], scalar2=None,
                                                op0=ALU.mult)
                        nc.vector.tensor_tensor(out=y3n[:, t, :], in0=y3n[:, t, :],
                                                in1=tmp[:], op=ALU.add)

            # ---------- pre0: q = A h0 (6-wide), then z0T = W0^T q, relu ----------
            w0_sb = main.tile([6, hid], F32, tag="w0_sb")
            nc.sync.dma_start(out=w0_sb[:], in_=w0[:, :])
            b0_sb = load_b_mid(b0)
            for t in range(nt):
                tp = t * P
                msg6 = msg6_pp[t % 2]
                edge_gather(msg6, t, h0[0:half, :], h0[half:, :], 64)
                q = accps([P, max(P, kc * P)])
                for j in range(KBT):
                    S = make_S(t * KBT + j)
                    nc.tensor.matmul(out=q[0:6, 0:P], lhsT=msg6[:, j, 0:6], rhs=S[:],
                                     start=(j == 0), stop=(j == KBT - 1))
                q_sb = smallp.tile([6, P], F32, tag="q_sb")
                nc.vector.tensor_copy(out=q_sb[:], in_=q[0:6, 0:P])
                for jj in range(kc):
                    z0 = densps([P, hid])
                    nc.tensor.matmul(out=z0[:, 0:P], lhsT=w0_sb[:, jj * P:(jj + 1) * P],
                                     rhs=q_sb[:], start=True, stop=True)
                    nc.scalar.activation(out=hT[:, jj, tp:tp + P], in_=z0[:, 0:P],
                                         func=AF.Relu, bias=b0_sb[:, jj:jj + 1])

            # ---------- middle layers ----------
            def dense_mid(w_sb, zsh_d, li):
                for t in range(nt):
                    tp = t * P
                    zps = densps([P, hid])
                    for k in range(kc):
                        nc.tensor.matmul(out=zps[:], lhsT=hT[:, k, tp:tp + P],
                                         rhs=w_sb[:, k, :], start=(k == 0),
                                         stop=(k == kc - 1) and li != 2)
                    if li == 2:
                        pt3 = tps([P, P])
                        nc.tensor.transpose(out=pt3[0:out_dim, 0:P],
                                            in_=y3n[:, t, :], identity=iden[:])
                        y3t_T = smallp.tile([out_dim, P], F32, tag="y3t_T")
                        nc.vector.tensor_copy(out=y3t_T[:], in_=pt3[0:out_dim, 0:P])
                        nc.tensor.matmul(out=zps[:], lhsT=y3t_T[:],
                                         rhs=wtop_sb[:, :], start=False, stop=True)
                    z_sb = zp.tile([P, hid], F32, tag="z_sb")
                    nc.scalar.activation(out=z_sb[:], in_=zps[:], func=AF.Copy)
                    nc.sync.dma_start(out=zsh_d[tp:tp + P, :], in_=z_sb[:])

            def sparse_mid(zfull_d, b_sb):
                for t in range(nt):
                    tp = t * P
                    msg = msg_pp[t % 2]
                    edge_gather(msg, t, zfull_d[0:half, :], zfull_d[half:, :], hid)
                    hps = accps([P, kc * P])
                    S_list = [make_S(t * KBT + j) for j in range(KBT)]
                    for cc in range(kc):
                        for j in range(KBT):
                            nc.tensor.matmul(out=hps[:, cc * P:(cc + 1) * P],
                                             lhsT=msg[:, j, cc * P:(cc + 1) * P],
                                             rhs=S_list[j][:], start=(j == 0),
                                             stop=(j == KBT - 1))
                    for cc in range(kc):
                        nc.scalar.activation(out=hT[:, cc, tp:tp + P],
                                             in_=hps[:, cc * P:(cc + 1) * P],
                                             func=AF.Relu, bias=b_sb[:, cc:cc + 1])

            for li in range(4):
                w_sb = load_w_mid(w_mid[li])
                b_sb = load_b_mid(b_mid[li])
                dense_mid(w_sb, zsh[li], li)
                nc.gpsimd.collective_compute(
                    "AllGather", ALU.bypass, replica_groups=rg,
                    ins=[zsh[li][:, :]], outs=[zfull[li][:, :]])
                sparse_mid(zfull[li], b_sb)

            # ---------- end2: z5T = W5^T h, transpose, AG, sparse3 + bias ----------
            w5_sb = main.tile([P, kc, out_dim], BF16, tag="w5_sb")
            nc.gpsimd.dma_start(out=w5_sb[:],
                                in_=w5[:, :].rearrange("(k p) o -> p k o", p=P))
            b5_sb = main.tile([out_dim, 1], F32, tag="b5_sb")
            nc.sync.dma_start(out=b5_sb[:], in_=b5[:, None])

            for t in range(nt):
                tp = t * P
                z5ps = densps([P, hid])
                for k in range(kc):
                    nc.tensor.matmul(out=z5ps[0:out_dim, 0:P], lhsT=w5_sb[:, k, :],
                                     rhs=hT[:, k, tp:tp + P], start=(k == 0),
                                     stop=(k == kc - 1))
                # transpose [3, 128] -> [128, 3] and store node-major
                z5T_sb = smallp.tile([out_dim, P], F32, tag="z5T_sb")
                nc.vector.tensor_copy(out=z5T_sb[:], in_=z5ps[0:out_dim, 0:P])
                ptp = tps([P, P])
                nc.tensor.transpose(out=ptp[:, 0:out_dim], in_=z5T_sb[:],
                                    identity=iden[0:out_dim, 0:out_dim])
                z5_sb = z5w_pp[t % 2]
                nc.vector.tensor_copy(out=z5_sb[:, 0:out_dim], in_=ptp[:, 0:out_dim])
                nc.sync.dma_start(out=z5sh[tp:tp + P, :], in_=z5_sb[:])
            nc.gpsimd.collective_compute(
                "AllGather", ALU.bypass, replica_groups=rg,
                ins=[z5sh[:, :]], outs=[z5full[:, :]])
            for t in range(nt):
                tp = t * P
                msg3 = msg6_pp[t % 2]
                edge_gather(msg3, t, z5full[0:half, :], z5full[half:, :], 64)
                ops = accps([P, max(P, kc * P)])
                for j in range(KBT):
                    S = make_S(t * KBT + j)
                    nc.tensor.matmul(out=ops[0:out_dim, 0:P],
                                     lhsT=msg3[:, j, 0:out_dim],
                                     rhs=S[:], start=(j == 0), stop=(j == KBT - 1))
                oT = smallp.tile([out_dim, P], F32, tag="oT")
                nc.vector.tensor_scalar(out=oT[:], in0=ops[0:out_dim, 0:P],
                                        scalar1=b5_sb[:, 0:1], scalar2=None,
                                        op0=ALU.add)
                po = tps([P, P])
                nc.tensor.transpose(out=po[:, 0:out_dim], in_=oT[:],
                                    identity=iden[0:out_dim, 0:out_dim])
                o_sb = smallp.tile([P, out_dim], F32, tag="o_sb")
                nc.vector.tensor_copy(out=o_sb[:], in_=po[:, 0:out_dim])
                nc.sync.dma_start(out=y_out[tp:tp + P, :], in_=o_sb[:])

    nc.finalize()
    return nc


# ---------------------------------------------------------------- entry point
def _prepare(inputs, n_fine, n_coarse, hid, out_dim, ncores):
    x = np.asarray(inputs["x"], np.float32)
    sdf = np.asarray(inputs["sdf"], np.float32)
    coarse_x = np.asarray(inputs["coarse_x"], np.float32)
    coarse_y = np.asarray(inputs["coarse_y"], np.float32)
    edge_index = np.asarray(inputs["edge_index"])

    KBA, KBB, nt, padsh, edges = _preprocess_edges(edge_index, n_fine, ncores)
    nsh = n_fine // ncores

    h0 = np.zeros((n_fine, 64), np.float32)
    h0[:, 0:5] = x
    h0[:, 5:6] = sdf
    h0pad = _pad_shard(h0, nsh, padsh, ncores)

    xpos = x[:, :2].astype(np.float32)
    xposT = []
    xpos_nm_l = []
    for c in range(ncores):
        xx = np.zeros((2, padsh), np.float32)
        xx[:, :nsh] = xpos[c * nsh:(c + 1) * nsh].T
        xposT.append(xx)
        xn = np.zeros((padsh, 2), np.float32)
        xn[:nsh] = xpos[c * nsh:(c + 1) * nsh]
        xpos_nm_l.append(xn)
    coarseT = np.ascontiguousarray(coarse_x[:, :2].T).astype(np.float32)

    in_maps = []
    for c in range(ncores):
        m = {
            "h0": h0pad,
            "idxA": edges[c]["idxA"], "idxB": edges[c]["idxB"],
            "cntAB": edges[c]["cntAB"],
            "ecol": edges[c]["ecol"], "enorm": edges[c]["enorm"],
            "xposT": xposT[c], "xpos_nm": xpos_nm_l[c],
            "coarseT": coarseT, "ycoarse": coarse_y,
            "w0": np.asarray(inputs["pre_W0"], np.float32),
            "b0": np.asarray(inputs["pre_b0"], np.float32),
            "w1": np.asarray(inputs["pre_W1"], np.float32),
            "b1": np.asarray(inputs["pre_b1"], np.float32),
            "w2": np.asarray(inputs["pre_W2"], np.float32),
            "b2": np.asarray(inputs["pre_b2"], np.float32),
            # end_W0 is [OUT+HID, HID]: top 3 rows couple y3, rest couple h
            "wtop": np.ascontiguousarray(np.asarray(inputs["end_W0"], np.float32)[:out_dim]),
            "we0": np.ascontiguousarray(np.asarray(inputs["end_W0"], np.float32)[out_dim:]),
            "be0": np.asarray(inputs["end_b0"], np.float32),
            "we1": np.asarray(inputs["end_W1"], np.float32),
            "be1": np.asarray(inputs["end_b1"], np.float32),
            "w5": np.asarray(inputs["end_W2"], np.float32),
            "b5": np.asarray(inputs["end_b2"], np.float32),
        }
        in_maps.append(m)
    return KBA, KBB, nt, padsh, in_maps


def run(inputs, n_fine=N_FINE, n_coarse=N_COARSE, hid=HID, out_dim=OUT,
        ncores=NCORES, sim=False, trace=False):
    KBA, KBB, nt, padsh, in_maps = _prepare(inputs, n_fine, n_coarse, hid,
                                            out_dim, ncores)
    key = (n_fine, n_coarse, hid, out_dim, ncores, KBA, KBB, nt)
    if key not in _PROGRAM_CACHE:
        _PROGRAM_CACHE[key] = build_program(n_fine, n_coarse, hid, out_dim,
                                            ncores, KBA, KBB, nt)
    nc = _PROGRAM_CACHE[key]

    nsh = n_fine // ncores
    if sim:
        from concourse.bass_interp import MultiCoreSim
        ms = MultiCoreSim(nc, ncores, num_workers=1)
        for c in range(ncores):
            for k, v in in_maps[c].items():
                ms.cores[c].tensor(k)[:] = v
        ms.simulate()
        outs = [np.array(ms.cores[c].tensor("out")) for c in range(ncores)]
        exec_ns = None
    else:
        from concourse.bass_utils import run_bass_kernel_spmd
        res = run_bass_kernel_spmd(nc, in_maps, list(range(ncores)), trace=trace)
        outs = [res.results[c]["out"] for c in range(ncores)]
        exec_ns = res.exec_time_ns

    full = np.zeros((n_fine, out_dim), np.float32)
    for c in range(ncores):
        full[c * nsh:(c + 1) * nsh] = outs[c][:nsh]
    return full, exec_ns


def kernel(**inputs):
    out, _ = run(inputs)
    return out



# revision 9
# speedup vs baseline: 1.5709x; 1.5709x over previous
"""CFD-GCN Trainium2 kernel: 6-layer GCN + KNN-interpolate on 8 NeuronCores.

v2 strategy (node sharding, feature-major residency, bf16 sparse path):
  - Fine nodes sharded 6250/core (padded 6272 = 49*128 = nt tiles).
  - Per GCN layer: z = h @ W (dense, bf16, PE) kept node-major in SBUF
    (znm) AND written to DRAM shards zshA/zshB split by local row range
    (A = rows 0:3200 / tiles 0-24, B = rows 3200:6272 / tiles 25-48).
    Two AllGathers per layer (A fires mid-dense, B at end) produce bf16
    tables zfullA [8*3200, 512] / zfullB [8*3072, 512] whose row spaces
    fit int16 gather indices with no core-half table split.
  - Edge gather: one dma_gather per (chunk, group of 3 dest tiles) with
    compile-time num_idxs (padded per tile to x128 with dummy idx 0 and
    norm 0 -- all indices valid, no count registers).
  - Scatter-add: one-hot S blocks (bf16, built on DVE from iota vs
    ecol/enorm) matmul'd against bf16 msg blocks into PSUM. Self-loops
    are NOT in the edge lists: a per-tile diagonal S block against the
    SBUF-resident node-major z supplies the dinv^2 * z term free of
    gather cost.
  - pre0 (A h0 then W0) and end2 (W5 then A z5) use the same index
    tables against 64-wide f32 tables h0A/h0B, z5fullA/z5fullB.
  - KNN-interpolate as before (matmul d2, DVE max8/max_index, small
    indirect gathers of coarse_y).
"""

import math
import numpy as np

# ---------------------------------------------------------------- constants
N_FINE = 50000
N_COARSE = 2000
HID = 512
OUT = 3
NCORES = 8
P = 128
CHA = 3200          # chunk A local rows (tiles 0..24)
CHB = 3072          # chunk B local rows (tiles 25..48)
TILES_A = CHA // P  # 25
GRP = 2             # dest tiles per gather group

_PROGRAM_CACHE = {}


# ---------------------------------------------------------------- host side
def _wrap16(flat, P=128):
    L = len(flat) // 16
    w = np.asarray(flat, np.int16).reshape(L, 16).T  # [16, L]
    return np.tile(w, (P // 16, 1))


def _preprocess_edges(edge_index, n_fine, ncores):
    """Dest-sorted edge lists, split by source-chunk (A/B local rows).

    Returns (KA, KB: per-tile block counts; per-core dict arrays).
    """
    nsh = n_fine // ncores              # 6250
    nt = math.ceil(nsh / P)             # 49
    padsh = nt * P                      # 6272

    row = np.asarray(edge_index[0]).astype(np.int64)
    col = np.asarray(edge_index[1]).astype(np.int64)

    deg = (np.bincount(col, minlength=n_fine) + 1.0).astype(np.float32)
    dinv = 1.0 / np.sqrt(deg)
    normv = (dinv[row] * dinv[col]).astype(np.float32)
    dinv2 = (dinv * dinv).astype(np.float32)

    order = np.argsort(col, kind="stable")
    col_s, row_s, norm_s = col[order], row[order], normv[order]

    src_core = row_s // nsh
    src_ls = row_s % nsh
    isa = src_ls < CHA
    idxA_val = src_core * CHA + src_ls
    idxB_val = src_core * CHB + (src_ls - CHA)

    # counts per (core, tile, chunk)
    cnt = np.zeros((ncores, nt, 2), np.int64)
    bounds = {}
    for c in range(ncores):
        base = c * nsh
        for t in range(nt):
            lo, hi = base + t * P, min(base + (t + 1) * P, base + nsh)
            a = np.searchsorted(col_s, lo, "left")
            b = np.searchsorted(col_s, hi, "left")
            na = int(isa[a:b].sum())
            cnt[c, t, 0] = na
            cnt[c, t, 1] = (b - a) - na
            bounds[(c, t)] = (a, b)

    KA = [int(math.ceil(max(1, cnt[:, t, 0].max()) / P)) for t in range(nt)]
    KB = [int(math.ceil(max(1, cnt[:, t, 1].max()) / P)) for t in range(nt)]
    totKA, totKB = sum(KA), sum(KB)
    offA = np.concatenate([[0], np.cumsum(KA)]) * P
    offB = np.concatenate([[0], np.cumsum(KB)]) * P
    # ecol/enorm block-column layout: per tile [selfloop | A blocks | B blocks]
    colbase = [t + (offA[t] + offB[t]) // P for t in range(nt)]
    nblk = nt + totKA + totKB

    out = []
    for c in range(ncores):
        flatA = np.zeros(totKA * P, np.int64)
        flatB = np.zeros(totKB * P, np.int64)
        ecol = np.full((P, nblk), -1.0, np.float32)
        enorm = np.zeros((P, nblk), np.float32)
        base = c * nsh
        for t in range(nt):
            a, b = bounds[(c, t)]
            m = isa[a:b]
            crel = (col_s[a:b] - (base + t * P)).astype(np.float32)
            nrm = norm_s[a:b]
            cb = colbase[t]
            # self-loop diagonal block
            nvalid = min(nsh - t * P, P)
            pp = np.arange(nvalid)
            ecol[pp, cb] = pp
            enorm[pp, cb] = dinv2[base + t * P: base + t * P + nvalid]
            for half, (ids, off, K, flat, bcol) in enumerate((
                    (idxA_val[a:b][m], offA[t], KA[t], flatA, cb + 1),
                    (idxB_val[a:b][~m], offB[t], KB[t], flatB, cb + 1 + KA[t]))):
                n = len(ids)
                flat[off: off + n] = ids
                s = np.arange(n)
                cc = crel[m] if half == 0 else crel[~m]
                nn = nrm[m] if half == 0 else nrm[~m]
                ecol[s % P, bcol + s // P] = cc
                enorm[s % P, bcol + s // P] = nn
        out.append({
            "idxA": _wrap16(flatA), "idxB": _wrap16(flatB),
            "ecol": ecol, "enorm": enorm,
        })
    return KA, KB, nt, padsh, out


def _pad_local(x, nsh, padsh):
    d = x.shape[1]
    o = np.zeros((padsh, d), x.dtype)
    o[:nsh] = x
    return o


# ---------------------------------------------------------------- device side
def build_program(n_fine, n_coarse, hid, out_dim, ncores, KA, KB, nt):
    import concourse.bass as bass
    import concourse.mybir as mybir
    from concourse.bacc import Bacc
    from concourse.tile import TileContext
    from concourse.masks import make_identity
    from contextlib import ExitStack

    F32 = mybir.dt.float32
    BF16 = mybir.dt.bfloat16
    I16 = mybir.dt.int16
    I32 = mybir.dt.int32
    padsh = nt * P
    kc = hid // P
    rg = [list(range(ncores))]
    AF = mybir.ActivationFunctionType
    ALU = mybir.AluOpType
    IOO = bass.IndirectOffsetOnAxis
    ncpad = math.ceil(n_coarse / 512) * 512
    ncc = math.ceil(n_coarse / 512)

    totKA, totKB = sum(KA), sum(KB)
    offA = [0]
    offB = [0]
    for t in range(nt):
        offA.append(offA[-1] + KA[t] * P)
        offB.append(offB[-1] + KB[t] * P)
    colbase = [t + (offA[t] + offB[t]) // P for t in range(nt)]
    nblk = nt + totKA + totKB
    rowsA, rowsB = ncores * CHA, ncores * CHB

    # gather groups: tiles [g0, g1) with per-chunk slot ranges
    groups = []
    t0 = 0
    while t0 < nt:
        t1 = min(t0 + GRP, nt)
        groups.append((t0, t1))
        t0 = t1
    GA = max(offA[t1] - offA[t0] for t0, t1 in groups) // P  # max A blocks/group
    GB = max(offB[t1] - offB[t0] for t0, t1 in groups) // P

    nc = Bacc(num_devices=ncores)

    # ---- kernel I/O (per core) ----
    h0A = nc.declare_dram_parameter("h0A", [rowsA, 64], F32, isOutput=False)
    h0B = nc.declare_dram_parameter("h0B", [rowsB, 64], F32, isOutput=False)
    h0nm_d = nc.declare_dram_parameter("h0nm", [padsh, 8], F32, isOutput=False)
    idxA = nc.declare_dram_parameter("idxA", [P, totKA * 8], I16, isOutput=False)
    idxB = nc.declare_dram_parameter("idxB", [P, totKB * 8], I16, isOutput=False)
    ecol = nc.declare_dram_parameter("ecol", [P, nblk], F32, isOutput=False)
    enorm = nc.declare_dram_parameter("enorm", [P, nblk], F32, isOutput=False)
    xposT = nc.declare_dram_parameter("xposT", [2, padsh], F32, isOutput=False)
    xpos_nm = nc.declare_dram_parameter("xpos_nm", [padsh, 2], F32, isOutput=False)
    coarseT = nc.declare_dram_parameter("coarseT", [2, n_coarse], F32, isOutput=False)
    ycoarse = nc.declare_dram_parameter("ycoarse", [n_coarse, out_dim], F32, isOutput=False)
    w_mid = [nc.declare_dram_parameter(n, [hid, hid], F32, isOutput=False)
             for n in ("w1", "w2", "we0", "we1")]
    b_mid = [nc.declare_dram_parameter(n, [hid], F32, isOutput=False)
             for n in ("b1", "b2", "be0", "be1")]
    w0 = nc.declare_dram_parameter("w0", [6, hid], F32, isOutput=False)
    b0 = nc.declare_dram_parameter("b0", [hid], F32, isOutput=False)
    wtop = nc.declare_dram_parameter("wtop", [out_dim, hid], F32, isOutput=False)
    w5 = nc.declare_dram_parameter("w5", [hid, out_dim], F32, isOutput=False)
    b5 = nc.declare_dram_parameter("b5", [out_dim], F32, isOutput=False)
    y_out = nc.declare_dram_parameter("out", [padsh, out_dim], F32, isOutput=True)

    # ---- internal DRAM ----
    zshA = [nc.dram_tensor(f"zshA{i}", [CHA, hid], BF16) for i in range(4)]
    zshB = [nc.dram_tensor(f"zshB{i}", [CHB, hid], BF16) for i in range(4)]
    zfullA = [nc.dram_tensor(f"zfullA{i}", [rowsA, hid], BF16, addr_space="Shared")
              for i in range(4)]
    zfullB = [nc.dram_tensor(f"zfullB{i}", [rowsB, hid], BF16, addr_space="Shared")
              for i in range(4)]
    z5shA = nc.dram_tensor("z5shA", [CHA, 64], F32)
    z5shB = nc.dram_tensor("z5shB", [CHB, 64], F32)
    z5fullA = nc.dram_tensor("z5fullA", [rowsA, 64], F32, addr_space="Shared")
    z5fullB = nc.dram_tensor("z5fullB", [rowsB, 64], F32, addr_space="Shared")

    with TileContext(nc) as tc:
        with ExitStack() as ctx:
            main = ctx.enter_context(tc.tile_pool(name="main", bufs=1))
            sp = ctx.enter_context(tc.tile_pool(name="sp", bufs=max(1 + KA[t] + KB[t] for t in range(nt)) + 4))
            mp = ctx.enter_context(tc.tile_pool(name="mp", bufs=2))
            zp = ctx.enter_context(tc.tile_pool(name="zp", bufs=2))
            smallp = ctx.enter_context(tc.tile_pool(name="smallp", bufs=2))
            ppA = ctx.enter_context(tc.tile_pool(name="ppA", bufs=2, space="PSUM"))
            ppB = ctx.enter_context(tc.tile_pool(name="ppB", bufs=2, space="PSUM"))
            ppC = ctx.enter_context(tc.tile_pool(name="ppC", bufs=2, space="PSUM"))

            def accps(shape):
                return ppA.tile(shape, F32, tag="acc", name="acc")

            def densps(shape):
                return ppB.tile(shape, F32, tag="dacc", name="dacc")

            def tps(shape):
                return ppC.tile(shape, F32, tag="tp", name="tp")

            # ---------- persistent tiles ----------
            hT = main.tile([P, kc, padsh], BF16, tag="hT")
            znm = main.tile([P, nt, hid], BF16, tag="znm")
            z5nm = main.tile([P, nt, 4], F32, tag="z5nm")
            h0nm = main.tile([P, nt, 8], F32, tag="h0nm")
            y3n = main.tile([P, nt, out_dim], F32, tag="y3n")
            iota_f = main.tile([P, P], F32, tag="iota_f")
            iden = main.tile([P, P], F32, tag="iden")
            idxA_sb = main.tile([P, totKA * 8], I16, tag="idxA_sb")
            idxB_sb = main.tile([P, totKB * 8], I16, tag="idxB_sb")
            ecol_sb = main.tile([P, nblk], F32, tag="ecol_sb")
            enorm_sb = main.tile([P, nblk], F32, tag="enorm_sb")
            wtop_sb = main.tile([out_dim, hid], F32, tag="wtop_sb")
            w0_sb = main.tile([6, hid], F32, tag="w0_sb")
            w_sb = [main.tile([P, kc, hid], BF16, tag=f"w_sb{i}", name=f"w_sb{i}") for i in range(4)]
            b_sb = [main.tile([P, kc], F32, tag=f"b_sb{i}", name=f"b_sb{i}") for i in range(4)]
            b0_sb = main.tile([P, kc], F32, tag="b0_sb")
            w5_sb = main.tile([P, kc, out_dim], BF16, tag="w5_sb")
            b5_sb = main.tile([out_dim, 1], F32, tag="b5_sb")

            nc.sync.dma_start(out=idxA_sb[:], in_=idxA[:, :])
            nc.sync.dma_start(out=idxB_sb[:], in_=idxB[:, :])
            nc.sync.dma_start(out=ecol_sb[:], in_=ecol[:, :])
            nc.sync.dma_start(out=enorm_sb[:], in_=enorm[:, :])
            nc.sync.dma_start(out=wtop_sb[:], in_=wtop[:, :])
            nc.sync.dma_start(out=w0_sb[:], in_=w0[:, :])
            nc.sync.dma_start(
                out=h0nm[:], in_=h0nm_d[:, :].rearrange("(t p) d -> p t d", p=P))
            for i in range(4):
                nc.gpsimd.dma_start(
                    out=w_sb[i][:],
                    in_=w_mid[i][:, :].rearrange("(k p) h -> p k h", p=P))
                nc.sync.dma_start(
                    out=b_sb[i][:], in_=b_mid[i][:].rearrange("(k p) -> p k", p=P))
            nc.sync.dma_start(out=b0_sb[:], in_=b0[:].rearrange("(k p) -> p k", p=P))
            nc.gpsimd.dma_start(
                out=w5_sb[:], in_=w5[:, :].rearrange("(k p) o -> p k o", p=P))
            nc.sync.dma_start(out=b5_sb[:], in_=b5[:, None])

            iota_i = smallp.tile([P, P], I32, tag="iota_i")
            nc.gpsimd.iota(out=iota_i[:], pattern=[[1, P]], base=0,
                           channel_multiplier=0)
            nc.vector.tensor_copy(out=iota_f[:], in_=iota_i[:])
            make_identity(nc, iden[:])

            def make_S(g, dt):
                S = sp.tile([P, P], dt, tag="Sb" if dt == BF16 else "Sf", name="S")
                nc.vector.tensor_scalar(
                    out=S[:], in0=iota_f[:],
                    scalar1=ecol_sb[:, g:g + 1], scalar2=enorm_sb[:, g:g + 1],
                    op0=ALU.is_equal, op1=ALU.mult)
                return S

            # ---------- KNN (independent; writes y3n) ----------
            with tc.tile_pool(name="knn", bufs=2) as kp:
                mones_sb = kp.tile([1, P], F32, tag="mones_sb", bufs=1)
                nc.vector.memset(mones_sb[:], -1.0)
                coarse3 = kp.tile([3, n_coarse], F32, tag="coarse3", bufs=1)
                with tc.tile_pool(name="knnprep", bufs=1) as kprep:
                    nc.sync.dma_start(out=coarse3[0:2, :], in_=coarseT[:, :])
                    pones = kprep.tile([2, 1], F32, tag="pones")
                    nc.vector.memset(pones[:], 1.0)
                    for i in range(ncc):
                        a, b = i * 512, min((i + 1) * 512, n_coarse)
                        sqc = kprep.tile([2, 512], F32, tag="sqc")
                        nc.vector.tensor_tensor(out=sqc[:, : b - a],
                                                in0=coarse3[0:2, a:b],
                                                in1=coarse3[0:2, a:b], op=ALU.mult)
                        ps = tps([P, 512])
                        nc.tensor.matmul(out=ps[0:1, : b - a], lhsT=pones[:],
                                         rhs=sqc[:, : b - a], start=True, stop=True)
                        csq = kprep.tile([1, 512], F32, tag="csq")
                        nc.vector.tensor_copy(out=csq[:, : b - a], in_=ps[0:1, : b - a])
                        nc.sync.dma_start(out=coarse3[2:3, a:b], in_=csq[:, : b - a])

                    xnm = kprep.tile([P, nt, 2], F32, tag="xnm")
                    nc.sync.dma_start(
                        out=xnm[:], in_=xpos_nm[:, :].rearrange("(t p) d -> p t d", p=P))
                    sqn = kprep.tile([P, nt, 2], F32, tag="sqn")
                    nc.vector.tensor_tensor(out=sqn[:], in0=xnm[:], in1=xnm[:],
                                            op=ALU.mult)
                    fsqneg = kp.tile([P, nt], F32, tag="fsqneg", bufs=1)
                    nc.vector.tensor_reduce(out=fsqneg[:], in_=sqn[:],
                                            axis=mybir.AxisListType.X, op=ALU.add,
                                            negate=True)

                for t in range(nt):
                    tp_ = t * P
                    xp_t = kp.tile([2, P], F32, tag="xp_t")
                    nc.sync.dma_start(out=xp_t[:], in_=xposT[:, tp_:tp_ + P])
                    lhsT3 = kp.tile([3, P], F32, tag="lhsT3")
                    nc.vector.tensor_scalar_mul(lhsT3[0:2, :], xp_t[:], 2.0)
                    nc.sync.dma_start(out=lhsT3[2:3, :], in_=mones_sb[:])

                    d2 = kp.tile([P, ncpad], F32, tag="d2", bufs=1)
                    for i in range(ncc):
                        a, b = i * 512, min((i + 1) * 512, n_coarse)
                        dps = densps([P, 512])
                        nc.tensor.matmul(out=dps[:, : b - a], lhsT=lhsT3[:],
                                         rhs=coarse3[:, a:b], start=True, stop=True)
                        nc.vector.tensor_scalar(out=d2[:, a:b], in0=dps[:, : b - a],
                                                scalar1=fsqneg[:, t:t + 1],
                                                scalar2=None, op0=ALU.add)
                    vals = kp.tile([P, 8], F32, tag="vals")
                    nc.vector.max(out=vals[:], in_=d2[:, 0:n_coarse])
                    idxs = kp.tile([P, 8], mybir.dt.uint32, tag="idxs")
                    nc.vector.max_index(out=idxs[:], in_max=vals[:],
                                        in_values=d2[:, 0:n_coarse])
                    wv = kp.tile([P, 3], F32, tag="wv")
                    nc.vector.tensor_scalar(out=wv[:], in0=vals[:, 0:3],
                                            scalar1=-1.0, scalar2=1e-16,
                                            op0=ALU.mult, op1=ALU.max)
                    nc.vector.reciprocal(out=wv[:], in_=wv[:])
                    wsum = kp.tile([P, 1], F32, tag="wsum")
                    nc.vector.tensor_reduce(out=wsum[:], in_=wv[:],
                                            axis=mybir.AxisListType.X, op=ALU.add)
                    nc.vector.reciprocal(out=wsum[:], in_=wsum[:])
                    nc.vector.tensor_scalar(out=wv[:], in0=wv[:],
                                            scalar1=wsum[:, 0:1], scalar2=None,
                                            op0=ALU.mult)
                    yg = kp.tile([P, 3, out_dim], F32, tag="yg")
                    for k3 in range(3):
                        nc.gpsimd.indirect_dma_start(
                            out=yg[:, k3, :], out_offset=None, in_=ycoarse[:, :],
                            in_offset=IOO(ap=idxs[:, k3:k3 + 1], axis=0))
                    tmp = kp.tile([P, out_dim], F32, tag="tmp")
                    nc.vector.tensor_scalar(out=y3n[:, t, :], in0=yg[:, 0, :],
                                            scalar1=wv[:, 0:1], scalar2=None,
                                            op0=ALU.mult)
                    for k in (1, 2):
                        nc.vector.tensor_scalar(out=tmp[:], in0=yg[:, k, :],
                                                scalar1=wv[:, k:k + 1], scalar2=None,
                                                op0=ALU.mult)
                        nc.vector.tensor_tensor(out=y3n[:, t, :], in0=y3n[:, t, :],
                                                in1=tmp[:], op=ALU.add)

            # ---------- gather helper ----------
            def gather_group(tabA, tabB, g0, g1, elem, dt, tag):
                nbA = (offA[g1] - offA[g0]) // P
                nbB = (offB[g1] - offB[g0]) // P
                mA = mp.tile([P, GA, elem], dt, tag=tag + "A", name=tag + "A", bufs=1 if elem == 64 else None)
                mB = mp.tile([P, GB, elem], dt, tag=tag + "B", name=tag + "B", bufs=1 if elem == 64 else None)
                nc.gpsimd.dma_gather(
                    mA[:, 0:nbA, :], tabA,
                    idxA_sb[:, offA[g0] // 16: offA[g1] // 16],
                    nbA * P, nbA * P, elem)
                nc.gpsimd.dma_gather(
                    mB[:, 0:nbB, :], tabB,
                    idxB_sb[:, offB[g0] // 16: offB[g1] // 16],
                    nbB * P, nbB * P, elem)
                return mA, mB

            # per-tile scatter matmuls: nrows output rows, contraction over
            # [selfloop | A blocks | B blocks]
            def scatter_tile(t, g0, mA, mB, self_lhsT, acc, nrows, dt, cchunks):
                cb = colbase[t]
                S_sl = make_S(cb, dt)
                SA = [make_S(cb + 1 + j, dt) for j in range(KA[t])]
                SB = [make_S(cb + 1 + KA[t] + j, dt) for j in range(KB[t])]
                bA = (offA[t] - offA[g0]) // P
                bB = (offB[t] - offB[g0]) // P
                for ci, (c0, c1) in enumerate(cchunks):
                    last = ci == len(cchunks) - 1
                    nc.tensor.matmul(out=acc[0:nrows, c0 * P:c0 * P + P] if nrows < P
                                     else acc[:, c0 * P:c0 * P + P],
                                     lhsT=self_lhsT(c0), rhs=S_sl[:],
                                     start=True, stop=False)
                    for j in range(KA[t]):
                        nc.tensor.matmul(
                            out=acc[0:nrows, c0 * P:c0 * P + P] if nrows < P
                            else acc[:, c0 * P:c0 * P + P],
                            lhsT=mA[:, bA + j, c0 * P:c0 * P + P] if nrows == P
                            else mA[:, bA + j, 0:nrows],
                            rhs=SA[j][:], start=False, stop=False)
                    for j in range(KB[t]):
                        nc.tensor.matmul(
                            out=acc[0:nrows, c0 * P:c0 * P + P] if nrows < P
                            else acc[:, c0 * P:c0 * P + P],
                            lhsT=mB[:, bB + j, c0 * P:c0 * P + P] if nrows == P
                            else mB[:, bB + j, 0:nrows],
                            rhs=SB[j][:], start=False, stop=(j == KB[t] - 1))

            # ---------- pre0: q = A h0 (6-wide), z0 = W0^T q, relu ----------
            for g0, g1 in groups:
                mA, mB = gather_group(h0A[:, :], h0B[:, :], g0, g1, 64, F32, "m6")
                for t in range(g0, g1):
                    tp_ = t * P
                    q = accps([P, P])
                    scatter_tile(t, g0, mA, mB,
                                 lambda c0: h0nm[:, t, 0:6],
                                 q, 6, F32, [(0, 1)])
                    q_sb = smallp.tile([6, P], F32, tag="q_sb")
                    nc.vector.tensor_copy(out=q_sb[:], in_=q[0:6, 0:P])
                    for jj in range(kc):
                        z0 = densps([P, P])
                        nc.tensor.matmul(out=z0[:, 0:P],
                                         lhsT=w0_sb[:, jj * P:(jj + 1) * P],
                                         rhs=q_sb[:], start=True, stop=True)
                        nc.scalar.activation(out=hT[:, jj, tp_:tp_ + P],
                                             in_=z0[:, 0:P], func=AF.Relu,
                                             bias=b0_sb[:, jj:jj + 1])

            # ---------- mid layers ----------
            def dense_mid(li):
                for t in range(nt):
                    tp_ = t * P
                    zps = densps([P, hid])
                    for k in range(kc):
                        nc.tensor.matmul(out=zps[:], lhsT=hT[:, k, tp_:tp_ + P],
                                         rhs=w_sb[li][:, k, :], start=(k == 0),
                                         stop=(k == kc - 1) and li != 2)
                    if li == 2:
                        pt3 = tps([P, P])
                        nc.tensor.transpose(out=pt3[0:out_dim, 0:P],
                                            in_=y3n[:, t, :], identity=iden[:])
                        y3t_T = smallp.tile([out_dim, P], F32, tag="y3t_T")
                        nc.vector.tensor_copy(out=y3t_T[:], in_=pt3[0:out_dim, 0:P])
                        nc.tensor.matmul(out=zps[:], lhsT=y3t_T[:],
                                         rhs=wtop_sb[:, :], start=False, stop=True)
                    nc.scalar.activation(out=znm[:, t, :], in_=zps[:], func=AF.Copy)
                    if t < TILES_A:
                        nc.sync.dma_start(out=zshA[li][tp_:tp_ + P, :],
                                          in_=znm[:, t, :])
                    else:
                        nc.sync.dma_start(
                            out=zshB[li][tp_ - CHA:tp_ - CHA + P, :],
                            in_=znm[:, t, :])
                    if t == TILES_A - 1:
                        nc.gpsimd.collective_compute(
                            "AllGather", ALU.bypass, replica_groups=rg,
                            ins=[zshA[li][:, :]], outs=[zfullA[li][:, :]])
                nc.gpsimd.collective_compute(
                    "AllGather", ALU.bypass, replica_groups=rg,
                    ins=[zshB[li][:, :]], outs=[zfullB[li][:, :]])

            def sparse_mid(li):
                for g0, g1 in groups:
                    mA, mB = gather_group(zfullA[li][:, :], zfullB[li][:, :],
                                          g0, g1, hid, BF16, "mm")
                    for t in range(g0, g1):
                        tp_ = t * P
                        hps = accps([P, kc * P])
                        scatter_tile(t, g0, mA, mB,
                                     lambda c0: znm[:, t, c0 * P:c0 * P + P],
                                     hps, P, BF16,
                                     [(k, k + 1) for k in range(kc)])
                        for cc in range(kc):
                            nc.scalar.activation(out=hT[:, cc, tp_:tp_ + P],
                                                 in_=hps[:, cc * P:(cc + 1) * P],
                                                 func=AF.Relu,
                                                 bias=b_sb[li][:, cc:cc + 1])

            for li in range(4):
                dense_mid(li)
                sparse_mid(li)

            # ---------- end2: z5 = W5^T h, AG, sparse + bias ----------
            for t in range(nt):
                tp_ = t * P
                z5ps = densps([P, hid])
                for k in range(kc):
                    nc.tensor.matmul(out=z5ps[0:out_dim, 0:P], lhsT=w5_sb[:, k, :],
                                     rhs=hT[:, k, tp_:tp_ + P], start=(k == 0),
                                     stop=(k == kc - 1))
                z5T_sb = smallp.tile([out_dim, P], F32, tag="z5T_sb")
                nc.vector.tensor_copy(out=z5T_sb[:], in_=z5ps[0:out_dim, 0:P])
                ptp = tps([P, P])
                nc.tensor.transpose(out=ptp[:, 0:out_dim], in_=z5T_sb[:],
                                    identity=iden[0:out_dim, 0:out_dim])
                nc.vector.tensor_copy(out=z5nm[:, t, 0:out_dim],
                                      in_=ptp[:, 0:out_dim])
                if t < TILES_A:
                    nc.sync.dma_start(out=z5shA[tp_:tp_ + P, 0:out_dim],
                                      in_=z5nm[:, t, 0:out_dim])
                else:
                    nc.sync.dma_start(
                        out=z5shB[tp_ - CHA:tp_ - CHA + P, 0:out_dim],
                        in_=z5nm[:, t, 0:out_dim])
                if t == TILES_A - 1:
                    nc.gpsimd.collective_compute(
                        "AllGather", ALU.bypass, replica_groups=rg,
                        ins=[z5shA[:, :]], outs=[z5fullA[:, :]])
            nc.gpsimd.collective_compute(
                "AllGather", ALU.bypass, replica_groups=rg,
                ins=[z5shB[:, :]], outs=[z5fullB[:, :]])

            for g0, g1 in groups:
                mA, mB = gather_group(z5fullA[:, :], z5fullB[:, :],
                                      g0, g1, 64, F32, "m6")
                for t in range(g0, g1):
                    tp_ = t * P
                    ops = accps([P, P])
                    scatter_tile(t, g0, mA, mB,
                                 lambda c0: z5nm[:, t, 0:out_dim],
                                 ops, out_dim, F32, [(0, 1)])
                    oT = smallp.tile([out_dim, P], F32, tag="oT")
                    nc.vector.tensor_scalar(out=oT[:], in0=ops[0:out_dim, 0:P],
                                            scalar1=b5_sb[:, 0:1], scalar2=None,
                                            op0=ALU.add)
                    po = tps([P, P])
                    nc.tensor.transpose(out=po[:, 0:out_dim], in_=oT[:],
                                        identity=iden[0:out_dim, 0:out_dim])
                    o_sb = smallp.tile([P, out_dim], F32, tag="o_sb")
                    nc.vector.tensor_copy(out=o_sb[:], in_=po[:, 0:out_dim])
                    nc.sync.dma_start(out=y_out[tp_:tp_ + P, :], in_=o_sb[:])

    nc.finalize()
    return nc


# ---------------------------------------------------------------- entry point
def _prepare(inputs, n_fine, n_coarse, hid, out_dim, ncores):
    x = np.asarray(inputs["x"], np.float32)
    sdf = np.asarray(inputs["sdf"], np.float32)
    coarse_x = np.asarray(inputs["coarse_x"], np.float32)
    coarse_y = np.asarray(inputs["coarse_y"], np.float32)
    edge_index = np.asarray(inputs["edge_index"])

    KA, KB, nt, padsh, edges = _preprocess_edges(edge_index, n_fine, ncores)
    nsh = n_fine // ncores

    h0 = np.zeros((n_fine, 64), np.float32)
    h0[:, 0:5] = x
    h0[:, 5:6] = sdf
    # chunk tables: rows c*CHA + ls (ls<CHA) / c*CHB + (ls-CHA)
    h0A = np.zeros((ncores * CHA, 64), np.float32)
    h0B = np.zeros((ncores * CHB, 64), np.float32)
    for c in range(ncores):
        sh = h0[c * nsh:(c + 1) * nsh]          # [6250, 64]
        h0A[c * CHA:(c + 1) * CHA] = sh[:CHA]
        h0B[c * CHB:c * CHB + (nsh - CHA)] = sh[CHA:]

    xpos = x[:, :2].astype(np.float32)
    coarseT = np.ascontiguousarray(coarse_x[:, :2].T).astype(np.float32)

    in_maps = []
    for c in range(ncores):
        xx = np.zeros((2, padsh), np.float32)
        xx[:, :nsh] = xpos[c * nsh:(c + 1) * nsh].T
        xn = np.zeros((padsh, 2), np.float32)
        xn[:nsh] = xpos[c * nsh:(c + 1) * nsh]
        h0nm = np.zeros((padsh, 8), np.float32)
        h0nm[:nsh, 0:6] = h0[c * nsh:(c + 1) * nsh, 0:6]
        m = {
            "h0A": h0A, "h0B": h0B, "h0nm": h0nm,
            "idxA": edges[c]["idxA"], "idxB": edges[c]["idxB"],
            "ecol": edges[c]["ecol"], "enorm": edges[c]["enorm"],
            "xposT": xx, "xpos_nm": xn,
            "coarseT": coarseT, "ycoarse": coarse_y,
            "w0": np.asarray(inputs["pre_W0"], np.float32),
            "b0": np.asarray(inputs["pre_b0"], np.float32),
            "w1": np.asarray(inputs["pre_W1"], np.float32),
            "b1": np.asarray(inputs["pre_b1"], np.float32),
            "w2": np.asarray(inputs["pre_W2"], np.float32),
            "b2": np.asarray(inputs["pre_b2"], np.float32),
            "wtop": np.ascontiguousarray(np.asarray(inputs["end_W0"], np.float32)[:out_dim]),
            "we0": np.ascontiguousarray(np.asarray(inputs["end_W0"], np.float32)[out_dim:]),
            "be0": np.asarray(inputs["end_b0"], np.float32),
            "we1": np.asarray(inputs["end_W1"], np.float32),
            "be1": np.asarray(inputs["end_b1"], np.float32),
            "w5": np.asarray(inputs["end_W2"], np.float32),
            "b5": np.asarray(inputs["end_b2"], np.float32),
        }
        in_maps.append(m)
    return KA, KB, nt, padsh, in_maps


def run(inputs, n_fine=N_FINE, n_coarse=N_COARSE, hid=HID, out_dim=OUT,
        ncores=NCORES, sim=False, trace=False):
    KA, KB, nt, padsh, in_maps = _prepare(inputs, n_fine, n_coarse, hid,
                                          out_dim, ncores)
    key = (n_fine, n_coarse, hid, out_dim, ncores, tuple(KA), tuple(KB), nt)
    if key not in _PROGRAM_CACHE:
        _PROGRAM_CACHE[key] = build_program(n_fine, n_coarse, hid, out_dim,
                                            ncores, KA, KB, nt)
    nc = _PROGRAM_CACHE[key]

    nsh = n_fine // ncores
    if sim:
        from concourse.bass_interp import MultiCoreSim
        ms = MultiCoreSim(nc, ncores, num_workers=1)
        for c in range(ncores):
            for k, v in in_maps[c].items():
                ms.cores[c].tensor(k)[:] = v
        ms.simulate()
        outs = [np.array(ms.cores[c].tensor("out")) for c in range(ncores)]
        exec_ns = None
    else:
        from concourse.bass_utils import run_bass_kernel_spmd
        res = run_bass_kernel_spmd(nc, in_maps, list(range(ncores)), trace=trace)
        outs = [res.results[c]["out"] for c in range(ncores)]
        exec_ns = res.exec_time_ns

    full = np.zeros((n_fine, out_dim), np.float32)
    for c in range(ncores):
        full[c * nsh:(c + 1) * nsh] = outs[c][:nsh]
    return full, exec_ns


def kernel(**inputs):
    out, _ = run(inputs)
    return out


# revision 10
# speedup vs baseline: 1.8012x; 1.1466x over previous
"""CFD-GCN Trainium2 kernel: 6-layer GCN + KNN-interpolate on 8 NeuronCores.

v3 strategy (node sharding, feature-major residency, bf16 sparse path):
  - Fine nodes sharded 6250/core (padded 6272 = 49*128 = nt tiles).
  - Per GCN layer: z = h @ W (dense, bf16, PE) kept node-major in SBUF
    (znm) AND written to DRAM shards zshA/zshB split by local row range
    (A = rows 0:3200 / tiles 0-24, B = rows 3200:6272 / tiles 25-48).
    The dense matmuls for layer l+1 are interleaved per-tile into the
    sparse phase of layer l, so AllGather-A fires ~halfway through the
    sparse phase and AllGather-B at its end -- both mostly hidden.
    Tables zfullA [8*3200, 512] / zfullB [8*3072, 512] keep row spaces
    within int16 gather-index range.
  - Edge gather: one dma_gather per (chunk, group of 2 dest tiles) with
    compile-time num_idxs (per-tile slots = max-over-cores count padded
    to x128 with dummy idx 0), no count registers.
  - Scatter-add: one-hot S blocks precomputed on the HOST as dense bf16
    [128 x 128] blocks (layout per tile [selfloop | A | B]) streamed
    from DRAM -- no on-device S construction. Self-loops are not in the
    edge lists: the diagonal selfloop block multiplies the SBUF-resident
    node-major z (dinv^2 * z) at zero gather cost.
  - pre0 (A h0 then W0) and end2 (W5 then A z5) run the same sparse
    machinery in bf16 against 128-wide tables h0A/h0B, z5fullA/z5fullB,
    reusing the same index tables and S blocks.
  - KNN-interpolate: matmul d2, DVE max8/max_index, small indirect
    gathers of coarse_y; overlaps the pre0 phase.
"""

import math
import numpy as np

# ---------------------------------------------------------------- constants
N_FINE = 50000
N_COARSE = 2000
HID = 512
OUT = 3
NCORES = 8
P = 128
CHA = 3200          # chunk A local rows (tiles 0..24)
CHB = 3072          # chunk B local rows (tiles 25..48)
TILES_A = CHA // P  # 25
GRP = 2             # dest tiles per gather group

_PROGRAM_CACHE = {}


# ---------------------------------------------------------------- host side
def _wrap16(flat, P=128):
    L = len(flat) // 16
    w = np.asarray(flat, np.int16).reshape(L, 16).T  # [16, L]
    return np.tile(w, (P // 16, 1))


def _preprocess_edges(edge_index, n_fine, ncores):
    """Dest-sorted edge lists split by source chunk + host-built S blocks."""
    import ml_dtypes
    bf16 = ml_dtypes.bfloat16
    nsh = n_fine // ncores              # 6250
    nt = math.ceil(nsh / P)             # 49
    padsh = nt * P                      # 6272

    row = np.asarray(edge_index[0]).astype(np.int64)
    col = np.asarray(edge_index[1]).astype(np.int64)

    deg = (np.bincount(col, minlength=n_fine) + 1.0).astype(np.float32)
    dinv = 1.0 / np.sqrt(deg)
    normv = (dinv[row] * dinv[col]).astype(np.float32)
    dinv2 = (dinv * dinv).astype(np.float32)

    order = np.argsort(col, kind="stable")
    col_s, row_s, norm_s = col[order], row[order], normv[order]

    src_core = row_s // nsh
    src_ls = row_s % nsh
    isa = src_ls < CHA
    idxA_val = src_core * CHA + src_ls
    idxB_val = src_core * CHB + (src_ls - CHA)

    cnt = np.zeros((ncores, nt, 2), np.int64)
    bounds = {}
    for c in range(ncores):
        base = c * nsh
        for t in range(nt):
            lo, hi = base + t * P, min(base + (t + 1) * P, base + nsh)
            a = np.searchsorted(col_s, lo, "left")
            b = np.searchsorted(col_s, hi, "left")
            na = int(isa[a:b].sum())
            cnt[c, t, 0] = na
            cnt[c, t, 1] = (b - a) - na
            bounds[(c, t)] = (a, b)

    KA = [int(math.ceil(max(1, cnt[:, t, 0].max()) / P)) for t in range(nt)]
    KB = [int(math.ceil(max(1, cnt[:, t, 1].max()) / P)) for t in range(nt)]
    totKA, totKB = sum(KA), sum(KB)
    offA = np.concatenate([[0], np.cumsum(KA)]) * P
    offB = np.concatenate([[0], np.cumsum(KB)]) * P
    colbase = [t + (offA[t] + offB[t]) // P for t in range(nt)]
    nblk = nt + totKA + totKB

    dvec = np.arange(P, dtype=np.float32)
    out = []
    for c in range(ncores):
        flatA = np.zeros(totKA * P, np.int64)
        flatB = np.zeros(totKB * P, np.int64)
        ecol = np.full((P, nblk), -1.0, np.float32)
        enorm = np.zeros((P, nblk), np.float32)
        base = c * nsh
        for t in range(nt):
            a, b = bounds[(c, t)]
            m = isa[a:b]
            crel = (col_s[a:b] - (base + t * P)).astype(np.float32)
            nrm = norm_s[a:b]
            cb = colbase[t]
            nvalid = min(nsh - t * P, P)
            pp = np.arange(nvalid)
            ecol[pp, cb] = pp
            enorm[pp, cb] = dinv2[base + t * P: base + t * P + nvalid]
            for half, (ids, off, flat, bcol) in enumerate((
                    (idxA_val[a:b][m], offA[t], flatA, cb + 1),
                    (idxB_val[a:b][~m], offB[t], flatB, cb + 1 + KA[t]))):
                n = len(ids)
                flat[off: off + n] = ids
                s = np.arange(n)
                cc = crel[m] if half == 0 else crel[~m]
                nn = nrm[m] if half == 0 else nrm[~m]
                ecol[s % P, bcol + s // P] = cc
                enorm[s % P, bcol + s // P] = nn
        sblk = ((ecol[:, :, None] == dvec[None, None, :])
                * enorm[:, :, None]).astype(bf16).reshape(P, nblk * P)
        out.append({
            "idxA": _wrap16(flatA), "idxB": _wrap16(flatB),
            "sblk": sblk,
        })
    return KA, KB, nt, padsh, out


# ---------------------------------------------------------------- device side
def build_program(n_fine, n_coarse, hid, out_dim, ncores, KA, KB, nt):
    import concourse.bass as bass
    import concourse.mybir as mybir
    from concourse.bacc import Bacc
    from concourse.tile import TileContext
    from concourse.masks import make_identity
    from contextlib import ExitStack

    F32 = mybir.dt.float32
    BF16 = mybir.dt.bfloat16
    I16 = mybir.dt.int16
    padsh = nt * P
    kc = hid // P
    rg = [list(range(ncores))]
    AF = mybir.ActivationFunctionType
    ALU = mybir.AluOpType
    IOO = bass.IndirectOffsetOnAxis
    ncpad = math.ceil(n_coarse / 512) * 512
    ncc = math.ceil(n_coarse / 512)

    totKA, totKB = sum(KA), sum(KB)
    offA = [0]
    offB = [0]
    for t in range(nt):
        offA.append(offA[-1] + KA[t] * P)
        offB.append(offB[-1] + KB[t] * P)
    colbase = [t + (offA[t] + offB[t]) // P for t in range(nt)]
    nblk = nt + totKA + totKB
    rowsA, rowsB = ncores * CHA, ncores * CHB

    groups = []
    t0 = 0
    while t0 < nt:
        t1 = min(t0 + GRP, nt)
        groups.append((t0, t1))
        t0 = t1
    GA = max(offA[t1] - offA[t0] for t0, t1 in groups) // P
    GB = max(offB[t1] - offB[t0] for t0, t1 in groups) // P

    nc = Bacc(num_devices=ncores)

    # ---- kernel I/O (per core) ----
    h0A = nc.declare_dram_parameter("h0A", [rowsA, P], BF16, isOutput=False)
    h0B = nc.declare_dram_parameter("h0B", [rowsB, P], BF16, isOutput=False)
    h0nm_d = nc.declare_dram_parameter("h0nm", [padsh, 8], BF16, isOutput=False)
    idxA = nc.declare_dram_parameter("idxA", [P, totKA * 8], I16, isOutput=False)
    idxB = nc.declare_dram_parameter("idxB", [P, totKB * 8], I16, isOutput=False)
    sblk = nc.declare_dram_parameter("sblk", [P, nblk * P], BF16, isOutput=False)
    xposT = nc.declare_dram_parameter("xposT", [2, padsh], F32, isOutput=False)
    xpos_nm = nc.declare_dram_parameter("xpos_nm", [padsh, 2], F32, isOutput=False)
    coarseT = nc.declare_dram_parameter("coarseT", [2, n_coarse], F32, isOutput=False)
    ycoarse = nc.declare_dram_parameter("ycoarse", [n_coarse, out_dim], F32, isOutput=False)
    w_mid = [nc.declare_dram_parameter(n, [hid, hid], F32, isOutput=False)
             for n in ("w1", "w2", "we0", "we1")]
    b_mid = [nc.declare_dram_parameter(n, [hid], F32, isOutput=False)
             for n in ("b1", "b2", "be0", "be1")]
    w0 = nc.declare_dram_parameter("w0", [6, hid], F32, isOutput=False)
    b0 = nc.declare_dram_parameter("b0", [hid], F32, isOutput=False)
    wtop = nc.declare_dram_parameter("wtop", [out_dim, hid], F32, isOutput=False)
    w5 = nc.declare_dram_parameter("w5", [hid, out_dim], F32, isOutput=False)
    b5 = nc.declare_dram_parameter("b5", [out_dim], F32, isOutput=False)
    y_out = nc.declare_dram_parameter("out", [padsh, out_dim], F32, isOutput=True)

    # ---- internal DRAM ----
    zshA = [nc.dram_tensor(f"zshA{i}", [CHA, hid], BF16) for i in range(4)]
    zshB = [nc.dram_tensor(f"zshB{i}", [CHB, hid], BF16) for i in range(4)]
    zfullA = [nc.dram_tensor(f"zfullA{i}", [rowsA, hid], BF16, addr_space="Shared")
              for i in range(4)]
    zfullB = [nc.dram_tensor(f"zfullB{i}", [rowsB, hid], BF16, addr_space="Shared")
              for i in range(4)]
    z5shA = nc.dram_tensor("z5shA", [CHA, P], BF16)
    z5shB = nc.dram_tensor("z5shB", [CHB, P], BF16)
    z5fullA = nc.dram_tensor("z5fullA", [rowsA, P], BF16, addr_space="Shared")
    z5fullB = nc.dram_tensor("z5fullB", [rowsB, P], BF16, addr_space="Shared")

    with TileContext(nc) as tc:
        with ExitStack() as ctx:
            main = ctx.enter_context(tc.tile_pool(name="main", bufs=1))
            sp = ctx.enter_context(tc.tile_pool(name="sp", bufs=3))
            mp = ctx.enter_context(tc.tile_pool(name="mp", bufs=2))
            smallp = ctx.enter_context(tc.tile_pool(name="smallp", bufs=2))
            ppA = ctx.enter_context(tc.tile_pool(name="ppA", bufs=2, space="PSUM"))
            ppB = ctx.enter_context(tc.tile_pool(name="ppB", bufs=2, space="PSUM"))
            ppC = ctx.enter_context(tc.tile_pool(name="ppC", bufs=2, space="PSUM"))

            def accps(shape):
                return ppA.tile(shape, F32, tag="acc", name="acc")

            def densps(shape):
                return ppB.tile(shape, F32, tag="dacc", name="dacc")

            def tps(shape):
                return ppC.tile(shape, F32, tag="tp", name="tp")

            # ---------- persistent tiles ----------
            hT = main.tile([P, kc, padsh], BF16, tag="hT")
            znm = main.tile([P, nt, hid], BF16, tag="znm")
            z5nm = main.tile([P, nt, 4], BF16, tag="z5nm")
            h0nm = main.tile([P, nt, 8], BF16, tag="h0nm")
            y3n = main.tile([P, nt, out_dim], F32, tag="y3n")
            iden = main.tile([P, P], F32, tag="iden")
            idxA_sb = main.tile([P, totKA * 8], I16, tag="idxA_sb")
            idxB_sb = main.tile([P, totKB * 8], I16, tag="idxB_sb")
            wtop_sb = main.tile([out_dim, hid], F32, tag="wtop_sb")
            w0_sb = main.tile([6, hid], F32, tag="w0_sb")
            w_sb = [main.tile([P, kc, hid], BF16, tag=f"w_sb{i}", name=f"w_sb{i}")
                    for i in range(4)]
            b_sb = [main.tile([P, kc], F32, tag=f"b_sb{i}", name=f"b_sb{i}")
                    for i in range(4)]
            b0_sb = main.tile([P, kc], F32, tag="b0_sb")
            w5_sb = main.tile([P, kc, out_dim], BF16, tag="w5_sb")
            b5_sb = main.tile([out_dim, 1], F32, tag="b5_sb")

            nc.sync.dma_start(out=idxA_sb[:], in_=idxA[:, :])
            nc.sync.dma_start(out=idxB_sb[:], in_=idxB[:, :])
            nc.sync.dma_start(out=wtop_sb[:], in_=wtop[:, :])
            nc.sync.dma_start(out=w0_sb[:], in_=w0[:, :])
            nc.sync.dma_start(
                out=h0nm[:], in_=h0nm_d[:, :].rearrange("(t p) d -> p t d", p=P))
            for i in range(4):
                nc.gpsimd.dma_start(
                    out=w_sb[i][:],
                    in_=w_mid[i][:, :].rearrange("(k p) h -> p k h", p=P))
                nc.sync.dma_start(
                    out=b_sb[i][:], in_=b_mid[i][:].rearrange("(k p) -> p k", p=P))
            nc.sync.dma_start(out=b0_sb[:], in_=b0[:].rearrange("(k p) -> p k", p=P))
            nc.gpsimd.dma_start(
                out=w5_sb[:], in_=w5[:, :].rearrange("(k p) o -> p k o", p=P))
            nc.sync.dma_start(out=b5_sb[:], in_=b5[:, None])
            make_identity(nc, iden[:])

            # ---------- KNN (independent; writes y3n) ----------
            with tc.tile_pool(name="knn", bufs=2) as kp:
                mones_sb = kp.tile([1, P], F32, tag="mones_sb", bufs=1)
                nc.vector.memset(mones_sb[:], -1.0)
                coarse3 = kp.tile([3, n_coarse], F32, tag="coarse3", bufs=1)
                with tc.tile_pool(name="knnprep", bufs=1) as kprep:
                    nc.sync.dma_start(out=coarse3[0:2, :], in_=coarseT[:, :])
                    pones = kprep.tile([2, 1], F32, tag="pones")
                    nc.vector.memset(pones[:], 1.0)
                    for i in range(ncc):
                        a, b = i * 512, min((i + 1) * 512, n_coarse)
                        sqc = kprep.tile([2, 512], F32, tag="sqc")
                        nc.vector.tensor_tensor(out=sqc[:, : b - a],
                                                in0=coarse3[0:2, a:b],
                                                in1=coarse3[0:2, a:b], op=ALU.mult)
                        ps = tps([P, 512])
                        nc.tensor.matmul(out=ps[0:1, : b - a], lhsT=pones[:],
                                         rhs=sqc[:, : b - a], start=True, stop=True)
                        csq = kprep.tile([1, 512], F32, tag="csq")
                        nc.vector.tensor_copy(out=csq[:, : b - a],
                                              in_=ps[0:1, : b - a])
                        nc.sync.dma_start(out=coarse3[2:3, a:b],
                                          in_=csq[:, : b - a])

                    xnm = kprep.tile([P, nt, 2], F32, tag="xnm")
                    nc.sync.dma_start(
                        out=xnm[:], in_=xpos_nm[:, :].rearrange("(t p) d -> p t d", p=P))
                    sqn = kprep.tile([P, nt, 2], F32, tag="sqn")
                    nc.vector.tensor_tensor(out=sqn[:], in0=xnm[:], in1=xnm[:],
                                            op=ALU.mult)
                    fsqneg = kp.tile([P, nt], F32, tag="fsqneg", bufs=1)
                    nc.vector.tensor_reduce(out=fsqneg[:], in_=sqn[:],
                                            axis=mybir.AxisListType.X, op=ALU.add,
                                            negate=True)

                for t in range(nt):
                    tp_ = t * P
                    xp_t = kp.tile([2, P], F32, tag="xp_t")
                    nc.sync.dma_start(out=xp_t[:], in_=xposT[:, tp_:tp_ + P])
                    lhsT3 = kp.tile([3, P], F32, tag="lhsT3")
                    nc.vector.tensor_scalar_mul(lhsT3[0:2, :], xp_t[:], 2.0)
                    nc.sync.dma_start(out=lhsT3[2:3, :], in_=mones_sb[:])

                    d2 = kp.tile([P, ncpad], F32, tag="d2", bufs=1)
                    for i in range(ncc):
                        a, b = i * 512, min((i + 1) * 512, n_coarse)
                        dps = densps([P, 512])
                        nc.tensor.matmul(out=dps[:, : b - a], lhsT=lhsT3[:],
                                         rhs=coarse3[:, a:b], start=True, stop=True)
                        nc.vector.tensor_scalar(out=d2[:, a:b], in0=dps[:, : b - a],
                                                scalar1=fsqneg[:, t:t + 1],
                                                scalar2=None, op0=ALU.add)
                    vals = kp.tile([P, 8], F32, tag="vals")
                    nc.vector.max(out=vals[:], in_=d2[:, 0:n_coarse])
                    idxs = kp.tile([P, 8], mybir.dt.uint32, tag="idxs")
                    nc.vector.max_index(out=idxs[:], in_max=vals[:],
                                        in_values=d2[:, 0:n_coarse])
                    wv = kp.tile([P, 3], F32, tag="wv")
                    nc.vector.tensor_scalar(out=wv[:], in0=vals[:, 0:3],
                                            scalar1=-1.0, scalar2=1e-16,
                                            op0=ALU.mult, op1=ALU.max)
                    nc.vector.reciprocal(out=wv[:], in_=wv[:])
                    wsum = kp.tile([P, 1], F32, tag="wsum")
                    nc.vector.tensor_reduce(out=wsum[:], in_=wv[:],
                                            axis=mybir.AxisListType.X, op=ALU.add)
                    nc.vector.reciprocal(out=wsum[:], in_=wsum[:])
                    nc.vector.tensor_scalar(out=wv[:], in0=wv[:],
                                            scalar1=wsum[:, 0:1], scalar2=None,
                                            op0=ALU.mult)
                    yg = kp.tile([P, 3, out_dim], F32, tag="yg")
                    for k3 in range(3):
                        nc.gpsimd.indirect_dma_start(
                            out=yg[:, k3, :], out_offset=None, in_=ycoarse[:, :],
                            in_offset=IOO(ap=idxs[:, k3:k3 + 1], axis=0))
                    tmp = kp.tile([P, out_dim], F32, tag="tmp")
                    nc.vector.tensor_scalar(out=y3n[:, t, :], in0=yg[:, 0, :],
                                            scalar1=wv[:, 0:1], scalar2=None,
                                            op0=ALU.mult)
                    for k in (1, 2):
                        nc.vector.tensor_scalar(out=tmp[:], in0=yg[:, k, :],
                                                scalar1=wv[:, k:k + 1], scalar2=None,
                                                op0=ALU.mult)
                        nc.vector.tensor_tensor(out=y3n[:, t, :], in0=y3n[:, t, :],
                                                in1=tmp[:], op=ALU.add)

            # ---------- helpers ----------
            def gather_group(tabA, tabB, g0, g1, elem, tag):
                nbA = (offA[g1] - offA[g0]) // P
                nbB = (offB[g1] - offB[g0]) // P
                mA = mp.tile([P, GA, elem], BF16, tag=tag + "A", name=tag + "A")
                mB = mp.tile([P, GB, elem], BF16, tag=tag + "B", name=tag + "B")
                nc.gpsimd.dma_gather(
                    mA[:, 0:nbA, :], tabA,
                    idxA_sb[:, offA[g0] // 16: offA[g1] // 16],
                    nbA * P, nbA * P, elem)
                nc.gpsimd.dma_gather(
                    mB[:, 0:nbB, :], tabB,
                    idxB_sb[:, offB[g0] // 16: offB[g1] // 16],
                    nbB * P, nbB * P, elem)
                return mA, mB

            def load_S(t):
                nbt = 1 + KA[t] + KB[t]
                St = sp.tile([P, nbt * P], BF16, tag="St", name="St")
                nc.sync.dma_start(
                    out=St[:], in_=sblk[:, colbase[t] * P:(colbase[t] + nbt) * P])
                return St

            def scatter_tile(t, g0, mA, mB, self_lhsT, acc, nrows, cchunks, St):
                bA = (offA[t] - offA[g0]) // P
                bB = (offB[t] - offB[g0]) // P
                for c0 in cchunks:
                    out = (acc[0:nrows, c0 * P:c0 * P + P] if nrows < P
                           else acc[:, c0 * P:c0 * P + P])
                    nc.tensor.matmul(out=out, lhsT=self_lhsT(c0),
                                     rhs=St[:, 0:P], start=True, stop=False)
                    for j in range(KA[t]):
                        nc.tensor.matmul(
                            out=out,
                            lhsT=(mA[:, bA + j, c0 * P:c0 * P + P] if nrows == P
                                  else mA[:, bA + j, 0:nrows]),
                            rhs=St[:, (1 + j) * P:(2 + j) * P],
                            start=False, stop=False)
                    for j in range(KB[t]):
                        nc.tensor.matmul(
                            out=out,
                            lhsT=(mB[:, bB + j, c0 * P:c0 * P + P] if nrows == P
                                  else mB[:, bB + j, 0:nrows]),
                            rhs=St[:, (1 + KA[t] + j) * P:(2 + KA[t] + j) * P],
                            start=False, stop=(j == KB[t] - 1))

            def dense_tile(li, t):
                # z_{li} = h @ W_li for tile t -> znm + zsh; AGs fired at
                # chunk boundaries so they overlap the remaining sparse work
                tp_ = t * P
                zps = densps([P, hid])
                for k in range(kc):
                    nc.tensor.matmul(out=zps[:], lhsT=hT[:, k, tp_:tp_ + P],
                                     rhs=w_sb[li][:, k, :], start=(k == 0),
                                     stop=(k == kc - 1) and li != 2)
                if li == 2:
                    pt3 = tps([P, P])
                    nc.tensor.transpose(out=pt3[0:out_dim, 0:P],
                                        in_=y3n[:, t, :], identity=iden[:])
                    y3t_T = smallp.tile([out_dim, P], F32, tag="y3t_T")
                    nc.vector.tensor_copy(out=y3t_T[:], in_=pt3[0:out_dim, 0:P])
                    nc.tensor.matmul(out=zps[:], lhsT=y3t_T[:],
                                     rhs=wtop_sb[:, :], start=False, stop=True)
                nc.scalar.activation(out=znm[:, t, :], in_=zps[:], func=AF.Copy)
                if t < TILES_A:
                    nc.sync.dma_start(out=zshA[li][tp_:tp_ + P, :], in_=znm[:, t, :])
                else:
                    nc.sync.dma_start(out=zshB[li][tp_ - CHA:tp_ - CHA + P, :],
                                      in_=znm[:, t, :])
                if t == TILES_A - 1:
                    nc.gpsimd.collective_compute(
                        "AllGather", ALU.bypass, replica_groups=rg,
                        ins=[zshA[li][:, :]], outs=[zfullA[li][:, :]])
                if t == nt - 1:
                    nc.gpsimd.collective_compute(
                        "AllGather", ALU.bypass, replica_groups=rg,
                        ins=[zshB[li][:, :]], outs=[zfullB[li][:, :]])

            def z5dense_tile(t):
                tp_ = t * P
                z5ps = densps([P, hid])
                for k in range(kc):
                    nc.tensor.matmul(out=z5ps[0:out_dim, 0:P], lhsT=w5_sb[:, k, :],
                                     rhs=hT[:, k, tp_:tp_ + P], start=(k == 0),
                                     stop=(k == kc - 1))
                z5T_sb = smallp.tile([out_dim, P], F32, tag="z5T_sb")
                nc.vector.tensor_copy(out=z5T_sb[:], in_=z5ps[0:out_dim, 0:P])
                ptp = tps([P, P])
                nc.tensor.transpose(out=ptp[:, 0:out_dim], in_=z5T_sb[:],
                                    identity=iden[0:out_dim, 0:out_dim])
                nc.vector.tensor_copy(out=z5nm[:, t, 0:out_dim],
                                      in_=ptp[:, 0:out_dim])
                if t < TILES_A:
                    nc.sync.dma_start(out=z5shA[tp_:tp_ + P, 0:out_dim],
                                      in_=z5nm[:, t, 0:out_dim])
                else:
                    nc.sync.dma_start(out=z5shB[tp_ - CHA:tp_ - CHA + P, 0:out_dim],
                                      in_=z5nm[:, t, 0:out_dim])
                if t == TILES_A - 1:
                    nc.gpsimd.collective_compute(
                        "AllGather", ALU.bypass, replica_groups=rg,
                        ins=[z5shA[:, :]], outs=[z5fullA[:, :]])
                if t == nt - 1:
                    nc.gpsimd.collective_compute(
                        "AllGather", ALU.bypass, replica_groups=rg,
                        ins=[z5shB[:, :]], outs=[z5fullB[:, :]])

            # ---------- pre0: q = A h0 (6-wide), z0 = W0^T q, relu; dense0 ----
            for g0, g1 in groups:
                mA, mB = gather_group(h0A[:, :], h0B[:, :], g0, g1, P, "m6")
                for t in range(g0, g1):
                    tp_ = t * P
                    St = load_S(t)
                    q = accps([P, P])
                    scatter_tile(t, g0, mA, mB, lambda c0: h0nm[:, t, 0:6],
                                 q, 6, [0], St)
                    q_sb = smallp.tile([6, P], F32, tag="q_sb")
                    nc.vector.tensor_copy(out=q_sb[:], in_=q[0:6, 0:P])
                    for jj in range(kc):
                        z0 = densps([P, P])
                        nc.tensor.matmul(out=z0[:, 0:P],
                                         lhsT=w0_sb[:, jj * P:(jj + 1) * P],
                                         rhs=q_sb[:], start=True, stop=True)
                        nc.scalar.activation(out=hT[:, jj, tp_:tp_ + P],
                                             in_=z0[:, 0:P], func=AF.Relu,
                                             bias=b0_sb[:, jj:jj + 1])
                    dense_tile(0, t)

            # ---------- mid layers: sparse(li) + interleaved dense(li+1) ----
            for li in range(4):
                for g0, g1 in groups:
                    mA, mB = gather_group(zfullA[li][:, :], zfullB[li][:, :],
                                          g0, g1, hid, "mm")
                    for t in range(g0, g1):
                        tp_ = t * P
                        St = load_S(t)
                        hps = accps([P, kc * P])
                        scatter_tile(t, g0, mA, mB,
                                     lambda c0: znm[:, t, c0 * P:c0 * P + P],
                                     hps, P, list(range(kc)), St)
                        for cc in range(kc):
                            nc.scalar.activation(out=hT[:, cc, tp_:tp_ + P],
                                                 in_=hps[:, cc * P:(cc + 1) * P],
                                                 func=AF.Relu,
                                                 bias=b_sb[li][:, cc:cc + 1])
                        if li < 3:
                            dense_tile(li + 1, t)
                        else:
                            z5dense_tile(t)

            # ---------- end2 final: out = A z5 + b5 ----------
            for g0, g1 in groups:
                mA, mB = gather_group(z5fullA[:, :], z5fullB[:, :], g0, g1, P, "m6")
                for t in range(g0, g1):
                    tp_ = t * P
                    St = load_S(t)
                    ops = accps([P, P])
                    scatter_tile(t, g0, mA, mB, lambda c0: z5nm[:, t, 0:out_dim],
                                 ops, out_dim, [0], St)
                    oT = smallp.tile([out_dim, P], F32, tag="oT")
                    nc.vector.tensor_scalar(out=oT[:], in0=ops[0:out_dim, 0:P],
                                            scalar1=b5_sb[:, 0:1], scalar2=None,
                                            op0=ALU.add)
                    po = tps([P, P])
                    nc.tensor.transpose(out=po[:, 0:out_dim], in_=oT[:],
                                        identity=iden[0:out_dim, 0:out_dim])
                    o_sb = smallp.tile([P, out_dim], F32, tag="o_sb")
                    nc.vector.tensor_copy(out=o_sb[:], in_=po[:, 0:out_dim])
                    nc.sync.dma_start(out=y_out[tp_:tp_ + P, :], in_=o_sb[:])

    nc.finalize()
    return nc


# ---------------------------------------------------------------- entry point
def _prepare(inputs, n_fine, n_coarse, hid, out_dim, ncores):
    import ml_dtypes
    bf16 = ml_dtypes.bfloat16
    x = np.asarray(inputs["x"], np.float32)
    sdf = np.asarray(inputs["sdf"], np.float32)
    coarse_x = np.asarray(inputs["coarse_x"], np.float32)
    coarse_y = np.asarray(inputs["coarse_y"], np.float32)
    edge_index = np.asarray(inputs["edge_index"])

    KA, KB, nt, padsh, edges = _preprocess_edges(edge_index, n_fine, ncores)
    nsh = n_fine // ncores

    h0 = np.zeros((n_fine, P), np.float32)
    h0[:, 0:5] = x
    h0[:, 5:6] = sdf
    h0A = np.zeros((ncores * CHA, P), bf16)
    h0B = np.zeros((ncores * CHB, P), bf16)
    for c in range(ncores):
        sh = h0[c * nsh:(c + 1) * nsh]
        h0A[c * CHA:(c + 1) * CHA] = sh[:CHA].astype(bf16)
        h0B[c * CHB:c * CHB + (nsh - CHA)] = sh[CHA:].astype(bf16)

    xpos = x[:, :2].astype(np.float32)
    coarseT = np.ascontiguousarray(coarse_x[:, :2].T).astype(np.float32)

    in_maps = []
    for c in range(ncores):
        xx = np.zeros((2, padsh), np.float32)
        xx[:, :nsh] = xpos[c * nsh:(c + 1) * nsh].T
        xn = np.zeros((padsh, 2), np.float32)
        xn[:nsh] = xpos[c * nsh:(c + 1) * nsh]
        h0nm = np.zeros((padsh, 8), bf16)
        h0nm[:nsh, 0:6] = h0[c * nsh:(c + 1) * nsh, 0:6].astype(bf16)
        m = {
            "h0A": h0A, "h0B": h0B, "h0nm": h0nm,
            "idxA": edges[c]["idxA"], "idxB": edges[c]["idxB"],
            "sblk": edges[c]["sblk"],
            "xposT": xx, "xpos_nm": xn,
            "coarseT": coarseT, "ycoarse": coarse_y,
            "w0": np.asarray(inputs["pre_W0"], np.float32),
            "b0": np.asarray(inputs["pre_b0"], np.float32),
            "w1": np.asarray(inputs["pre_W1"], np.float32),
            "b1": np.asarray(inputs["pre_b1"], np.float32),
            "w2": np.asarray(inputs["pre_W2"], np.float32),
            "b2": np.asarray(inputs["pre_b2"], np.float32),
            "wtop": np.ascontiguousarray(np.asarray(inputs["end_W0"], np.float32)[:out_dim]),
            "we0": np.ascontiguousarray(np.asarray(inputs["end_W0"], np.float32)[out_dim:]),
            "be0": np.asarray(inputs["end_b0"], np.float32),
            "we1": np.asarray(inputs["end_W1"], np.float32),
            "be1": np.asarray(inputs["end_b1"], np.float32),
            "w5": np.asarray(inputs["end_W2"], np.float32),
            "b5": np.asarray(inputs["end_b2"], np.float32),
        }
        in_maps.append(m)
    return KA, KB, nt, padsh, in_maps


def run(inputs, n_fine=N_FINE, n_coarse=N_COARSE, hid=HID, out_dim=OUT,
        ncores=NCORES, sim=False, trace=False):
    KA, KB, nt, padsh, in_maps = _prepare(inputs, n_fine, n_coarse, hid,
                                          out_dim, ncores)
    key = (n_fine, n_coarse, hid, out_dim, ncores, tuple(KA), tuple(KB), nt)
    if key not in _PROGRAM_CACHE:
        _PROGRAM_CACHE[key] = build_program(n_fine, n_coarse, hid, out_dim,
                                            ncores, KA, KB, nt)
    nc = _PROGRAM_CACHE[key]

    nsh = n_fine // ncores
    if sim:
        from concourse.bass_interp import MultiCoreSim
        ms = MultiCoreSim(nc, ncores, num_workers=1)
        for c in range(ncores):
            for k, v in in_maps[c].items():
                ms.cores[c].tensor(k)[:] = v
        ms.simulate()
        outs = [np.array(ms.cores[c].tensor("out")) for c in range(ncores)]
        exec_ns = None
    else:
        from concourse.bass_utils import run_bass_kernel_spmd
        res = run_bass_kernel_spmd(nc, in_maps, list(range(ncores)), trace=trace)
        outs = [res.results[c]["out"] for c in range(ncores)]
        exec_ns = res.exec_time_ns

    full = np.zeros((n_fine, out_dim), np.float32)
    for c in range(ncores):
        full[c * nsh:(c + 1) * nsh] = outs[c][:nsh]
    return full, exec_ns


def kernel(**inputs):
    out, _ = run(inputs)
    return out


# revision 12
# speedup vs baseline: 1.9301x; 1.0716x over previous
"""CFD-GCN Trainium2 kernel: 6-layer GCN + KNN-interpolate on 8 NeuronCores.

v3 strategy (node sharding, feature-major residency, bf16 sparse path):
  - Fine nodes sharded 6250/core (padded 6272 = 49*128 = nt tiles).
  - Per GCN layer: z = h @ W (dense, bf16, PE) kept node-major in SBUF
    (znm) AND written to DRAM shards zshA/zshB split by local row range
    (A = rows 0:3200 / tiles 0-24, B = rows 3200:6272 / tiles 25-48).
    The dense matmuls for layer l+1 are interleaved per-tile into the
    sparse phase of layer l, so AllGather-A fires ~halfway through the
    sparse phase and AllGather-B at its end -- both mostly hidden.
    Tables zfullA [8*3200, 512] / zfullB [8*3072, 512] keep row spaces
    within int16 gather-index range.
  - Edge gather: one dma_gather per (chunk, group of 2 dest tiles) with
    compile-time num_idxs (per-tile slots = max-over-cores count padded
    to x128 with dummy idx 0), no count registers.
  - Scatter-add: one-hot S blocks precomputed on the HOST as dense bf16
    [128 x 128] blocks (layout per tile [selfloop | A | B]) streamed
    from DRAM -- no on-device S construction. Self-loops are not in the
    edge lists: the diagonal selfloop block multiplies the SBUF-resident
    node-major z (dinv^2 * z) at zero gather cost.
  - pre0 (A h0 then W0) and end2 (W5 then A z5) run the same sparse
    machinery in bf16 against 128-wide tables h0A/h0B, z5fullA/z5fullB,
    reusing the same index tables and S blocks.
  - KNN-interpolate: matmul d2, DVE max8/max_index, small indirect
    gathers of coarse_y; overlaps the pre0 phase.
"""

import math
import numpy as np

# ---------------------------------------------------------------- constants
N_FINE = 50000
N_COARSE = 2000
HID = 512
OUT = 3
NCORES = 8
P = 128
CHA = 3200          # chunk A local rows (tiles 0..24)
CHB = 3072          # chunk B local rows (tiles 25..48)
TILES_A = CHA // P  # 25
GRP = 2             # dest tiles per gather group

_PROGRAM_CACHE = {}


# ---------------------------------------------------------------- host side
def _wrap16(flat, P=128):
    L = len(flat) // 16
    w = np.asarray(flat, np.int16).reshape(L, 16).T  # [16, L]
    return np.tile(w, (P // 16, 1))


def _preprocess_edges(edge_index, n_fine, ncores):
    """Dest-sorted edge lists split by source chunk + host-built S blocks."""
    import ml_dtypes
    bf16 = ml_dtypes.bfloat16
    nsh = n_fine // ncores              # 6250
    nt = math.ceil(nsh / P)             # 49
    padsh = nt * P                      # 6272

    row = np.asarray(edge_index[0]).astype(np.int64)
    col = np.asarray(edge_index[1]).astype(np.int64)

    deg = (np.bincount(col, minlength=n_fine) + 1.0).astype(np.float32)
    dinv = 1.0 / np.sqrt(deg)
    normv = (dinv[row] * dinv[col]).astype(np.float32)
    dinv2 = (dinv * dinv).astype(np.float32)

    order = np.argsort(col, kind="stable")
    col_s, row_s, norm_s = col[order], row[order], normv[order]

    src_core = row_s // nsh
    src_ls = row_s % nsh
    isa = src_ls < CHA
    idxA_val = src_core * CHA + src_ls
    idxB_val = src_core * CHB + (src_ls - CHA)

    cnt = np.zeros((ncores, nt, 2), np.int64)
    bounds = {}
    for c in range(ncores):
        base = c * nsh
        for t in range(nt):
            lo, hi = base + t * P, min(base + (t + 1) * P, base + nsh)
            a = np.searchsorted(col_s, lo, "left")
            b = np.searchsorted(col_s, hi, "left")
            na = int(isa[a:b].sum())
            cnt[c, t, 0] = na
            cnt[c, t, 1] = (b - a) - na
            bounds[(c, t)] = (a, b)

    KA = [int(math.ceil(max(1, cnt[:, t, 0].max()) / P)) for t in range(nt)]
    KB = [int(math.ceil(max(1, cnt[:, t, 1].max()) / P)) for t in range(nt)]
    totKA, totKB = sum(KA), sum(KB)
    offA = np.concatenate([[0], np.cumsum(KA)]) * P
    offB = np.concatenate([[0], np.cumsum(KB)]) * P
    colbase = [t + (offA[t] + offB[t]) // P for t in range(nt)]
    nblk = nt + totKA + totKB

    dvec = np.arange(P, dtype=np.float32)
    out = []
    for c in range(ncores):
        flatA = np.zeros(totKA * P, np.int64)
        flatB = np.zeros(totKB * P, np.int64)
        ecol = np.full((P, nblk), -1.0, np.float32)
        enorm = np.zeros((P, nblk), np.float32)
        base = c * nsh
        for t in range(nt):
            a, b = bounds[(c, t)]
            m = isa[a:b]
            crel = (col_s[a:b] - (base + t * P)).astype(np.float32)
            nrm = norm_s[a:b]
            cb = colbase[t]
            nvalid = min(nsh - t * P, P)
            pp = np.arange(nvalid)
            ecol[pp, cb] = pp
            enorm[pp, cb] = dinv2[base + t * P: base + t * P + nvalid]
            for half, (ids, off, flat, bcol) in enumerate((
                    (idxA_val[a:b][m], offA[t], flatA, cb + 1),
                    (idxB_val[a:b][~m], offB[t], flatB, cb + 1 + KA[t]))):
                n = len(ids)
                flat[off: off + n] = ids
                s = np.arange(n)
                cc = crel[m] if half == 0 else crel[~m]
                nn = nrm[m] if half == 0 else nrm[~m]
                ecol[s % P, bcol + s // P] = cc
                enorm[s % P, bcol + s // P] = nn
        sblk = ((ecol[:, :, None] == dvec[None, None, :])
                * enorm[:, :, None]).astype(bf16).reshape(P, nblk * P)
        out.append({
            "idxA": _wrap16(flatA), "idxB": _wrap16(flatB),
            "sblk": sblk,
        })
    return KA, KB, nt, padsh, out


# ---------------------------------------------------------------- device side
def build_program(n_fine, n_coarse, hid, out_dim, ncores, KA, KB, nt):
    import concourse.bass as bass
    import concourse.mybir as mybir
    from concourse.bacc import Bacc
    from concourse.tile import TileContext
    from concourse.masks import make_identity
    from contextlib import ExitStack

    F32 = mybir.dt.float32
    BF16 = mybir.dt.bfloat16
    I16 = mybir.dt.int16
    padsh = nt * P
    kc = hid // P
    rg = [list(range(ncores))]
    AF = mybir.ActivationFunctionType
    ALU = mybir.AluOpType
    IOO = bass.IndirectOffsetOnAxis
    ncpad = math.ceil(n_coarse / 512) * 512
    ncc = math.ceil(n_coarse / 512)

    totKA, totKB = sum(KA), sum(KB)
    offA = [0]
    offB = [0]
    for t in range(nt):
        offA.append(offA[-1] + KA[t] * P)
        offB.append(offB[-1] + KB[t] * P)
    colbase = [t + (offA[t] + offB[t]) // P for t in range(nt)]
    nblk = nt + totKA + totKB
    rowsA, rowsB = ncores * CHA, ncores * CHB

    groups = []
    t0 = 0
    while t0 < nt:
        t1 = min(t0 + GRP, nt)
        groups.append((t0, t1))
        t0 = t1
    GA = max(offA[t1] - offA[t0] for t0, t1 in groups) // P
    GB = max(offB[t1] - offB[t0] for t0, t1 in groups) // P

    nc = Bacc(num_devices=ncores)

    # ---- kernel I/O (per core) ----
    h0A = nc.declare_dram_parameter("h0A", [rowsA, P], BF16, isOutput=False)
    h0B = nc.declare_dram_parameter("h0B", [rowsB, P], BF16, isOutput=False)
    h0nm_d = nc.declare_dram_parameter("h0nm", [padsh, 8], BF16, isOutput=False)
    idxA = nc.declare_dram_parameter("idxA", [P, totKA * 8], I16, isOutput=False)
    idxB = nc.declare_dram_parameter("idxB", [P, totKB * 8], I16, isOutput=False)
    sblk = nc.declare_dram_parameter("sblk", [P, nblk * P], BF16, isOutput=False)
    xposT = nc.declare_dram_parameter("xposT", [2, padsh], F32, isOutput=False)
    xpos_nm = nc.declare_dram_parameter("xpos_nm", [padsh, 2], F32, isOutput=False)
    coarseT = nc.declare_dram_parameter("coarseT", [2, n_coarse], F32, isOutput=False)
    ycoarse = nc.declare_dram_parameter("ycoarse", [n_coarse, out_dim], F32, isOutput=False)
    w_mid = [nc.declare_dram_parameter(n, [hid, hid], F32, isOutput=False)
             for n in ("w1", "w2", "we0", "we1")]
    b_mid = [nc.declare_dram_parameter(n, [hid], F32, isOutput=False)
             for n in ("b1", "b2", "be0", "be1")]
    w0 = nc.declare_dram_parameter("w0", [6, hid], F32, isOutput=False)
    b0 = nc.declare_dram_parameter("b0", [hid], F32, isOutput=False)
    wtop = nc.declare_dram_parameter("wtop", [out_dim, hid], F32, isOutput=False)
    w5 = nc.declare_dram_parameter("w5", [hid, out_dim], F32, isOutput=False)
    b5 = nc.declare_dram_parameter("b5", [out_dim], F32, isOutput=False)
    y_out = nc.declare_dram_parameter("out", [padsh, out_dim], F32, isOutput=True)

    # ---- internal DRAM ----
    zshA = [nc.dram_tensor(f"zshA{i}", [CHA, hid], BF16) for i in range(4)]
    zshB = [nc.dram_tensor(f"zshB{i}", [CHB, hid], BF16) for i in range(4)]
    zfullA = [nc.dram_tensor(f"zfullA{i}", [rowsA, hid], BF16, addr_space="Shared")
              for i in range(4)]
    zfullB = [nc.dram_tensor(f"zfullB{i}", [rowsB, hid], BF16, addr_space="Shared")
              for i in range(4)]
    z5shA = nc.dram_tensor("z5shA", [CHA, P], BF16)
    z5shB = nc.dram_tensor("z5shB", [CHB, P], BF16)
    z5fullA = nc.dram_tensor("z5fullA", [rowsA, P], BF16, addr_space="Shared")
    z5fullB = nc.dram_tensor("z5fullB", [rowsB, P], BF16, addr_space="Shared")

    with TileContext(nc) as tc:
        with ExitStack() as ctx:
            main = ctx.enter_context(tc.tile_pool(name="main", bufs=1))
            sp = ctx.enter_context(tc.tile_pool(name="sp", bufs=3))
            mp = ctx.enter_context(tc.tile_pool(name="mp", bufs=2))
            smallp = ctx.enter_context(tc.tile_pool(name="smallp", bufs=2))
            ppA = ctx.enter_context(tc.tile_pool(name="ppA", bufs=2, space="PSUM"))
            ppB = ctx.enter_context(tc.tile_pool(name="ppB", bufs=2, space="PSUM"))
            ppC = ctx.enter_context(tc.tile_pool(name="ppC", bufs=2, space="PSUM"))

            def accps(shape):
                return ppA.tile(shape, F32, tag="acc", name="acc")

            def densps(shape):
                return ppB.tile(shape, F32, tag="dacc", name="dacc")

            def tps(shape):
                return ppC.tile(shape, F32, tag="tp", name="tp")

            # ---------- persistent tiles ----------
            hT = main.tile([P, kc, padsh], BF16, tag="hT")
            znm = main.tile([P, nt, hid], BF16, tag="znm")
            z5nm = main.tile([P, nt, 4], BF16, tag="z5nm")
            h0nm = main.tile([P, nt, 8], BF16, tag="h0nm")
            y3n = main.tile([P, nt, out_dim], F32, tag="y3n")
            iden = main.tile([P, P], F32, tag="iden")
            idxA_sb = main.tile([P, totKA * 8], I16, tag="idxA_sb")
            idxB_sb = main.tile([P, totKB * 8], I16, tag="idxB_sb")
            wtop_sb = main.tile([out_dim, hid], F32, tag="wtop_sb")
            w0_sb = main.tile([6, hid], F32, tag="w0_sb")
            w_sb = [main.tile([P, kc, hid], BF16, tag=f"w_sb{i}", name=f"w_sb{i}")
                    for i in range(4)]
            b_sb = [main.tile([P, kc], F32, tag=f"b_sb{i}", name=f"b_sb{i}")
                    for i in range(4)]
            b0_sb = main.tile([P, kc], F32, tag="b0_sb")
            w5_sb = main.tile([P, kc, out_dim], BF16, tag="w5_sb")
            b5_sb = main.tile([out_dim, 1], F32, tag="b5_sb")

            nc.sync.dma_start(out=idxA_sb[:], in_=idxA[:, :])
            nc.sync.dma_start(out=idxB_sb[:], in_=idxB[:, :])
            nc.sync.dma_start(out=wtop_sb[:], in_=wtop[:, :])
            nc.sync.dma_start(out=w0_sb[:], in_=w0[:, :])
            nc.sync.dma_start(
                out=h0nm[:], in_=h0nm_d[:, :].rearrange("(t p) d -> p t d", p=P))
            for i in range(4):
                nc.gpsimd.dma_start(
                    out=w_sb[i][:],
                    in_=w_mid[i][:, :].rearrange("(k p) h -> p k h", p=P))
                nc.sync.dma_start(
                    out=b_sb[i][:], in_=b_mid[i][:].rearrange("(k p) -> p k", p=P))
            nc.sync.dma_start(out=b0_sb[:], in_=b0[:].rearrange("(k p) -> p k", p=P))
            nc.gpsimd.dma_start(
                out=w5_sb[:], in_=w5[:, :].rearrange("(k p) o -> p k o", p=P))
            nc.sync.dma_start(out=b5_sb[:], in_=b5[:, None])
            make_identity(nc, iden[:])

            # ---------- helpers ----------
            def gather_group(tabA, tabB, g0, g1, elem, tag):
                nbA = (offA[g1] - offA[g0]) // P
                nbB = (offB[g1] - offB[g0]) // P
                mA = mp.tile([P, GA, elem], BF16, tag=tag + "A", name=tag + "A", bufs=None)
                mB = mp.tile([P, GB, elem], BF16, tag=tag + "B", name=tag + "B", bufs=None)
                nc.gpsimd.dma_gather(
                    mA[:, 0:nbA, :], tabA,
                    idxA_sb[:, offA[g0] // 16: offA[g1] // 16],
                    nbA * P, nbA * P, elem)
                nc.gpsimd.dma_gather(
                    mB[:, 0:nbB, :], tabB,
                    idxB_sb[:, offB[g0] // 16: offB[g1] // 16],
                    nbB * P, nbB * P, elem)
                return mA, mB

            def load_S(t):
                nbt = 1 + KA[t] + KB[t]
                St = sp.tile([P, nbt * P], BF16, tag="St", name="St")
                nc.sync.dma_start(
                    out=St[:], in_=sblk[:, colbase[t] * P:(colbase[t] + nbt) * P])
                return St

            def scatter_tile(t, g0, mA, mB, self_lhsT, acc, nrows, cchunks, St):
                bA = (offA[t] - offA[g0]) // P
                bB = (offB[t] - offB[g0]) // P
                for c0 in cchunks:
                    out = (acc[0:nrows, c0 * P:c0 * P + P] if nrows < P
                           else acc[:, c0 * P:c0 * P + P])
                    nc.tensor.matmul(out=out, lhsT=self_lhsT(c0),
                                     rhs=St[:, 0:P], start=True, stop=False)
                    for j in range(KA[t]):
                        nc.tensor.matmul(
                            out=out,
                            lhsT=(mA[:, bA + j, c0 * P:c0 * P + P] if nrows == P
                                  else mA[:, bA + j, 0:nrows]),
                            rhs=St[:, (1 + j) * P:(2 + j) * P],
                            start=False, stop=False)
                    for j in range(KB[t]):
                        nc.tensor.matmul(
                            out=out,
                            lhsT=(mB[:, bB + j, c0 * P:c0 * P + P] if nrows == P
                                  else mB[:, bB + j, 0:nrows]),
                            rhs=St[:, (1 + KA[t] + j) * P:(2 + KA[t] + j) * P],
                            start=False, stop=(j == KB[t] - 1))

            def dense_tile(li, t):
                # z_{li} = h @ W_li for tile t -> znm + zsh; AGs fired at
                # chunk boundaries so they overlap the remaining sparse work
                tp_ = t * P
                zps = densps([P, hid])
                for k in range(kc):
                    nc.tensor.matmul(out=zps[:], lhsT=hT[:, k, tp_:tp_ + P],
                                     rhs=w_sb[li][:, k, :], start=(k == 0),
                                     stop=(k == kc - 1) and li != 2)
                if li == 2:
                    pt3 = tps([P, P])
                    nc.tensor.transpose(out=pt3[0:out_dim, 0:P],
                                        in_=y3n[:, t, :], identity=iden[:])
                    y3t_T = smallp.tile([out_dim, P], F32, tag="y3t_T")
                    nc.vector.tensor_copy(out=y3t_T[:], in_=pt3[0:out_dim, 0:P])
                    nc.tensor.matmul(out=zps[:], lhsT=y3t_T[:],
                                     rhs=wtop_sb[:, :], start=False, stop=True)
                nc.scalar.activation(out=znm[:, t, :], in_=zps[:], func=AF.Copy)
                if t < TILES_A:
                    nc.sync.dma_start(out=zshA[li][tp_:tp_ + P, :], in_=znm[:, t, :])
                else:
                    nc.sync.dma_start(out=zshB[li][tp_ - CHA:tp_ - CHA + P, :],
                                      in_=znm[:, t, :])
                if t == TILES_A - 1:
                    nc.gpsimd.collective_compute(
                        "AllGather", ALU.bypass, replica_groups=rg,
                        ins=[zshA[li][:, :]], outs=[zfullA[li][:, :]])
                if t == nt - 1:
                    nc.gpsimd.collective_compute(
                        "AllGather", ALU.bypass, replica_groups=rg,
                        ins=[zshB[li][:, :]], outs=[zfullB[li][:, :]])

            def z5dense_tile(t):
                tp_ = t * P
                z5ps = densps([P, hid])
                for k in range(kc):
                    nc.tensor.matmul(out=z5ps[0:out_dim, 0:P], lhsT=w5_sb[:, k, :],
                                     rhs=hT[:, k, tp_:tp_ + P], start=(k == 0),
                                     stop=(k == kc - 1))
                z5T_sb = smallp.tile([out_dim, P], F32, tag="z5T_sb")
                nc.vector.tensor_copy(out=z5T_sb[:], in_=z5ps[0:out_dim, 0:P])
                ptp = tps([P, P])
                nc.tensor.transpose(out=ptp[:, 0:out_dim], in_=z5T_sb[:],
                                    identity=iden[0:out_dim, 0:out_dim])
                nc.vector.tensor_copy(out=z5nm[:, t, 0:out_dim],
                                      in_=ptp[:, 0:out_dim])
                if t < TILES_A:
                    nc.sync.dma_start(out=z5shA[tp_:tp_ + P, 0:out_dim],
                                      in_=z5nm[:, t, 0:out_dim])
                else:
                    nc.sync.dma_start(out=z5shB[tp_ - CHA:tp_ - CHA + P, 0:out_dim],
                                      in_=z5nm[:, t, 0:out_dim])
                if t == TILES_A - 1:
                    nc.gpsimd.collective_compute(
                        "AllGather", ALU.bypass, replica_groups=rg,
                        ins=[z5shA[:, :]], outs=[z5fullA[:, :]])
                if t == nt - 1:
                    nc.gpsimd.collective_compute(
                        "AllGather", ALU.bypass, replica_groups=rg,
                        ins=[z5shB[:, :]], outs=[z5fullB[:, :]])

            # ---------- pre0: q = A h0 (6-wide), z0 = W0^T q, relu; dense0 ----
            for g0, g1 in groups:
                mA, mB = gather_group(h0A[:, :], h0B[:, :], g0, g1, P, "m6")
                for t in range(g0, g1):
                    tp_ = t * P
                    St = load_S(t)
                    q = accps([P, P])
                    scatter_tile(t, g0, mA, mB, lambda c0: h0nm[:, t, 0:6],
                                 q, 6, [0], St)
                    q_sb = smallp.tile([6, P], F32, tag="q_sb")
                    nc.vector.tensor_copy(out=q_sb[:], in_=q[0:6, 0:P])
                    for jj in range(kc):
                        z0 = densps([P, P])
                        nc.tensor.matmul(out=z0[:, 0:P],
                                         lhsT=w0_sb[:, jj * P:(jj + 1) * P],
                                         rhs=q_sb[:], start=True, stop=True)
                        nc.scalar.activation(out=hT[:, jj, tp_:tp_ + P],
                                             in_=z0[:, 0:P], func=AF.Relu,
                                             bias=b0_sb[:, jj:jj + 1])
                    dense_tile(0, t)

            # ---------- KNN (independent; writes y3n) ----------
            with tc.tile_pool(name="knn", bufs=2) as kp:
                mones_sb = kp.tile([1, P], F32, tag="mones_sb", bufs=1)
                nc.vector.memset(mones_sb[:], -1.0)
                coarse3 = kp.tile([3, n_coarse], F32, tag="coarse3", bufs=1)
                with tc.tile_pool(name="knnprep", bufs=1) as kprep:
                    nc.sync.dma_start(out=coarse3[0:2, :], in_=coarseT[:, :])
                    pones = kprep.tile([2, 1], F32, tag="pones")
                    nc.vector.memset(pones[:], 1.0)
                    for i in range(ncc):
                        a, b = i * 512, min((i + 1) * 512, n_coarse)
                        sqc = kprep.tile([2, 512], F32, tag="sqc")
                        nc.vector.tensor_tensor(out=sqc[:, : b - a],
                                                in0=coarse3[0:2, a:b],
                                                in1=coarse3[0:2, a:b], op=ALU.mult)
                        ps = tps([P, 512])
                        nc.tensor.matmul(out=ps[0:1, : b - a], lhsT=pones[:],
                                         rhs=sqc[:, : b - a], start=True, stop=True)
                        csq = kprep.tile([1, 512], F32, tag="csq")
                        nc.vector.tensor_copy(out=csq[:, : b - a],
                                              in_=ps[0:1, : b - a])
                        nc.sync.dma_start(out=coarse3[2:3, a:b],
                                          in_=csq[:, : b - a])

                    xnm = kprep.tile([P, nt, 2], F32, tag="xnm")
                    nc.sync.dma_start(
                        out=xnm[:], in_=xpos_nm[:, :].rearrange("(t p) d -> p t d", p=P))
                    sqn = kprep.tile([P, nt, 2], F32, tag="sqn")
                    nc.vector.tensor_tensor(out=sqn[:], in0=xnm[:], in1=xnm[:],
                                            op=ALU.mult)
                    fsqneg = kp.tile([P, nt], F32, tag="fsqneg", bufs=1)
                    nc.vector.tensor_reduce(out=fsqneg[:], in_=sqn[:],
                                            axis=mybir.AxisListType.X, op=ALU.add,
                                            negate=True)

                for t in range(nt):
                    tp_ = t * P
                    xp_t = kp.tile([2, P], F32, tag="xp_t")
                    nc.sync.dma_start(out=xp_t[:], in_=xposT[:, tp_:tp_ + P])
                    lhsT3 = kp.tile([3, P], F32, tag="lhsT3")
                    nc.vector.tensor_scalar_mul(lhsT3[0:2, :], xp_t[:], 2.0)
                    nc.sync.dma_start(out=lhsT3[2:3, :], in_=mones_sb[:])

                    d2 = kp.tile([P, ncpad], F32, tag="d2", bufs=1)
                    for i in range(ncc):
                        a, b = i * 512, min((i + 1) * 512, n_coarse)
                        dps = densps([P, 512])
                        nc.tensor.matmul(out=dps[:, : b - a], lhsT=lhsT3[:],
                                         rhs=coarse3[:, a:b], start=True, stop=True)
                        nc.vector.tensor_scalar(out=d2[:, a:b], in0=dps[:, : b - a],
                                                scalar1=fsqneg[:, t:t + 1],
                                                scalar2=None, op0=ALU.add)
                    vals = kp.tile([P, 8], F32, tag="vals")
                    nc.vector.max(out=vals[:], in_=d2[:, 0:n_coarse])
                    idxs = kp.tile([P, 8], mybir.dt.uint32, tag="idxs")
                    nc.vector.max_index(out=idxs[:], in_max=vals[:],
                                        in_values=d2[:, 0:n_coarse])
                    wv = kp.tile([P, 3], F32, tag="wv")
                    nc.vector.tensor_scalar(out=wv[:], in0=vals[:, 0:3],
                                            scalar1=-1.0, scalar2=1e-16,
                                            op0=ALU.mult, op1=ALU.max)
                    nc.vector.reciprocal(out=wv[:], in_=wv[:])
                    wsum = kp.tile([P, 1], F32, tag="wsum")
                    nc.vector.tensor_reduce(out=wsum[:], in_=wv[:],
                                            axis=mybir.AxisListType.X, op=ALU.add)
                    nc.vector.reciprocal(out=wsum[:], in_=wsum[:])
                    nc.vector.tensor_scalar(out=wv[:], in0=wv[:],
                                            scalar1=wsum[:, 0:1], scalar2=None,
                                            op0=ALU.mult)
                    yg = kp.tile([P, 3, out_dim], F32, tag="yg")
                    for k3 in range(3):
                        nc.gpsimd.indirect_dma_start(
                            out=yg[:, k3, :], out_offset=None, in_=ycoarse[:, :],
                            in_offset=IOO(ap=idxs[:, k3:k3 + 1], axis=0))
                    tmp = kp.tile([P, out_dim], F32, tag="tmp")
                    nc.vector.tensor_scalar(out=y3n[:, t, :], in0=yg[:, 0, :],
                                            scalar1=wv[:, 0:1], scalar2=None,
                                            op0=ALU.mult)
                    for k in (1, 2):
                        nc.vector.tensor_scalar(out=tmp[:], in0=yg[:, k, :],
                                                scalar1=wv[:, k:k + 1], scalar2=None,
                                                op0=ALU.mult)
                        nc.vector.tensor_tensor(out=y3n[:, t, :], in0=y3n[:, t, :],
                                                in1=tmp[:], op=ALU.add)

            # ---------- mid layers: sparse(li) + interleaved dense(li+1) ----
            for li in range(4):
                for g0, g1 in groups:
                    mA, mB = gather_group(zfullA[li][:, :], zfullB[li][:, :],
                                          g0, g1, hid, "mm")
                    for t in range(g0, g1):
                        tp_ = t * P
                        St = load_S(t)
                        hps = accps([P, kc * P])
                        scatter_tile(t, g0, mA, mB,
                                     lambda c0: znm[:, t, c0 * P:c0 * P + P],
                                     hps, P, list(range(kc)), St)
                        for cc in range(kc):
                            nc.scalar.activation(out=hT[:, cc, tp_:tp_ + P],
                                                 in_=hps[:, cc * P:(cc + 1) * P],
                                                 func=AF.Relu,
                                                 bias=b_sb[li][:, cc:cc + 1])
                        if li < 3:
                            dense_tile(li + 1, t)
                        else:
                            z5dense_tile(t)

            # ---------- end2 final: out = A z5 + b5 ----------
            for g0, g1 in groups:
                mA, mB = gather_group(z5fullA[:, :], z5fullB[:, :], g0, g1, P, "m6")
                for t in range(g0, g1):
                    tp_ = t * P
                    St = load_S(t)
                    ops = accps([P, P])
                    scatter_tile(t, g0, mA, mB, lambda c0: z5nm[:, t, 0:out_dim],
                                 ops, out_dim, [0], St)
                    oT = smallp.tile([out_dim, P], F32, tag="oT")
                    nc.vector.tensor_scalar(out=oT[:], in0=ops[0:out_dim, 0:P],
                                            scalar1=b5_sb[:, 0:1], scalar2=None,
                                            op0=ALU.add)
                    po = tps([P, P])
                    nc.tensor.transpose(out=po[:, 0:out_dim], in_=oT[:],
                                        identity=iden[0:out_dim, 0:out_dim])
                    o_sb = smallp.tile([P, out_dim], F32, tag="o_sb")
                    nc.vector.tensor_copy(out=o_sb[:], in_=po[:, 0:out_dim])
                    nc.sync.dma_start(out=y_out[tp_:tp_ + P, :], in_=o_sb[:])

    nc.finalize()
    return nc


# ---------------------------------------------------------------- entry point
def _prepare(inputs, n_fine, n_coarse, hid, out_dim, ncores):
    import ml_dtypes
    bf16 = ml_dtypes.bfloat16
    x = np.asarray(inputs["x"], np.float32)
    sdf = np.asarray(inputs["sdf"], np.float32)
    coarse_x = np.asarray(inputs["coarse_x"], np.float32)
    coarse_y = np.asarray(inputs["coarse_y"], np.float32)
    edge_index = np.asarray(inputs["edge_index"])

    KA, KB, nt, padsh, edges = _preprocess_edges(edge_index, n_fine, ncores)
    nsh = n_fine // ncores

    h0 = np.zeros((n_fine, P), np.float32)
    h0[:, 0:5] = x
    h0[:, 5:6] = sdf
    h0A = np.zeros((ncores * CHA, P), bf16)
    h0B = np.zeros((ncores * CHB, P), bf16)
    for c in range(ncores):
        sh = h0[c * nsh:(c + 1) * nsh]
        h0A[c * CHA:(c + 1) * CHA] = sh[:CHA].astype(bf16)
        h0B[c * CHB:c * CHB + (nsh - CHA)] = sh[CHA:].astype(bf16)

    xpos = x[:, :2].astype(np.float32)
    coarseT = np.ascontiguousarray(coarse_x[:, :2].T).astype(np.float32)

    in_maps = []
    for c in range(ncores):
        xx = np.zeros((2, padsh), np.float32)
        xx[:, :nsh] = xpos[c * nsh:(c + 1) * nsh].T
        xn = np.zeros((padsh, 2), np.float32)
        xn[:nsh] = xpos[c * nsh:(c + 1) * nsh]
        h0nm = np.zeros((padsh, 8), bf16)
        h0nm[:nsh, 0:6] = h0[c * nsh:(c + 1) * nsh, 0:6].astype(bf16)
        m = {
            "h0A": h0A, "h0B": h0B, "h0nm": h0nm,
            "idxA": edges[c]["idxA"], "idxB": edges[c]["idxB"],
            "sblk": edges[c]["sblk"],
            "xposT": xx, "xpos_nm": xn,
            "coarseT": coarseT, "ycoarse": coarse_y,
            "w0": np.asarray(inputs["pre_W0"], np.float32),
            "b0": np.asarray(inputs["pre_b0"], np.float32),
            "w1": np.asarray(inputs["pre_W1"], np.float32),
            "b1": np.asarray(inputs["pre_b1"], np.float32),
            "w2": np.asarray(inputs["pre_W2"], np.float32),
            "b2": np.asarray(inputs["pre_b2"], np.float32),
            "wtop": np.ascontiguousarray(np.asarray(inputs["end_W0"], np.float32)[:out_dim]),
            "we0": np.ascontiguousarray(np.asarray(inputs["end_W0"], np.float32)[out_dim:]),
            "be0": np.asarray(inputs["end_b0"], np.float32),
            "we1": np.asarray(inputs["end_W1"], np.float32),
            "be1": np.asarray(inputs["end_b1"], np.float32),
            "w5": np.asarray(inputs["end_W2"], np.float32),
            "b5": np.asarray(inputs["end_b2"], np.float32),
        }
        in_maps.append(m)
    return KA, KB, nt, padsh, in_maps


def run(inputs, n_fine=N_FINE, n_coarse=N_COARSE, hid=HID, out_dim=OUT,
        ncores=NCORES, sim=False, trace=False):
    KA, KB, nt, padsh, in_maps = _prepare(inputs, n_fine, n_coarse, hid,
                                          out_dim, ncores)
    key = (n_fine, n_coarse, hid, out_dim, ncores, tuple(KA), tuple(KB), nt)
    if key not in _PROGRAM_CACHE:
        _PROGRAM_CACHE[key] = build_program(n_fine, n_coarse, hid, out_dim,
                                            ncores, KA, KB, nt)
    nc = _PROGRAM_CACHE[key]

    nsh = n_fine // ncores
    if sim:
        from concourse.bass_interp import MultiCoreSim
        ms = MultiCoreSim(nc, ncores, num_workers=1)
        for c in range(ncores):
            for k, v in in_maps[c].items():
                ms.cores[c].tensor(k)[:] = v
        ms.simulate()
        outs = [np.array(ms.cores[c].tensor("out")) for c in range(ncores)]
        exec_ns = None
    else:
        from concourse.bass_utils import run_bass_kernel_spmd
        res = run_bass_kernel_spmd(nc, in_maps, list(range(ncores)), trace=trace)
        outs = [res.results[c]["out"] for c in range(ncores)]
        exec_ns = res.exec_time_ns

    full = np.zeros((n_fine, out_dim), np.float32)
    for c in range(ncores):
        full[c * nsh:(c + 1) * nsh] = outs[c][:nsh]
    return full, exec_ns


def kernel(**inputs):
    out, _ = run(inputs)
    return out
